# revision 41
# baseline (speedup 1.0000x reference)
"""Trainium2 Bass kernel for batched multi-head self-attention.

Problem: x [8, 1500, 768], 12 heads x 64 dims, torch-Linear style projections.
Strategy: data-parallel over batch (1 element per NeuronCore, 8 cores).

Per-core design (host pre-transposes everything; device does no transposes):
  - xT [768, 1500]: projections contract over d on the partition axis.
  - Q^T, K^T in [e, s] layout (pairs of heads per 128-partition chunk).
    K^T and V stay SBUF-resident; Q^T roundtrips through a DRAM scratch
    (each [head-pair, q-block] slice is consumed exactly once).
  - q-blocks are uniformly 512 wide; the last block overlaps the previous
    one (start S-512) so no padding or edge cases exist for S >= 512.
  - scores computed TRANSPOSED: scoresT[k, q] = K_h^T.T @ Q_h^T, two heads
    per PE pass via row tile_position packing (contraction is dh=64 only).
  - exp on ScalarE straight out of PSUM ([128,1024] two-bank spans), no max
    subtraction (scores ~ N(0,1): fp32-safe).
  - softmax denominators ride as a 65th all-ones column of V inside the ctx
    matmul (ctxT psum = 64 ctx rows + 1 sums row).
  - normalization: reciprocal of the sums row, partition-broadcast via a
    tiny DRAM roundtrip, multiplied in during the ctx psum eviction.
  - output projection consumes ctx_normT [e, s] directly; the bv/bo
    contribution is a constant row (softmax rows sum to 1) added on host.

All matmul operands are bfloat16 (1 cycle/row on the PE at any moving
size, half the DMA bytes / SBUF of f32r). PSUM accumulation stays fp32;
softmax denominators come from the same bf16 e values as the numerators,
so normalization is consistent. End-to-end rel err ~5.7e-3 vs fp32
(gate: 2e-2). Output is written bf16 and upcast on host.

Scheduling notes (sim-guided, CoreSim cost model; body ~302us sim):
  - only the 12 per-head ones-columns of V are memset (a full-V memset
    occupies DVE ~10us at startup and stalls the first q-block eviction);
  - Wq/Wk ship host-swizzled as [NE, 128, D] so each per-pair weight
    load is one contiguous DMA (column-block loads of a [D, D] matrix
    pay a 2x sub-512B-row DMA penalty); pair 0's weights preload ahead
    of the xT stream, whose first q-block alternates both DMA queues;
    later q-blocks load as single strided DMAs (descriptor generation
    costs ~500ns per dma_start per queue);
  - woT loads after xT on the same queue (first needed only by phase3);
  - normalization: reciprocal straight off the PSUM sums row (bf16),
    DRAM broadcast roundtrip + Pool-engine multiply in steady state
    (GpSimd cannot touch PSUM, so eviction copies stay on DVE); the
    body-tail unit instead broadcasts via a rank-1 PE matmul
    (ones[1,DH]^T @ rc -> PSUM) to dodge the shared-DMA-engine backlog,
    evicts ctx on the post-exp-idle ScalarE, and multiplies on DVE;
  - output evictions alternate DMA queues (gpsimd/sync); the tail
    q-block also alternates its eviction copies across ScalarE/DVE.
"""

import numpy as np
from contextlib import ExitStack

import concourse.bass as bass
import concourse.bacc as bacc
import concourse.tile as tile
from concourse import mybir
from concourse import bass_utils

F32 = mybir.dt.float32
F32R = mybir.dt.float32r
BF16 = mybir.dt.bfloat16
CDT = BF16               # compute dtype for matmul operands (1 cyc/row)
NPCDT = mybir.dt.np(CDT)
AF = mybir.ActivationFunctionType
OP = mybir.AluOpType

P = 128
D = 768
H = 12
DH = 64
NE = D // P          # 6 e-chunks (head pairs)
ND = D // P          # 6 d-chunks
SCALE = 0.125
S_FULL = 1500
QB = 512
EH = 384             # half of D for the V projection moving dim


def _chunks(total, size):
    out = []
    o = 0
    while o < total:
        out.append((o, min(size, total - o)))
        o += size
    return out


def _qblocks(S):
    """512-wide q-blocks; the last one is narrower (phase 2 handles qw<512
    with split exp instructions, whose overhead is ~zero on hardware)."""
    return _chunks(S, QB)


def build_attention(tc, ctx, xT, wqT, wkT, wvT, woT, bqs, ident, out, S,
                    reps=1,
                    hw_loop=False):
    """Emit the single-core attention program.

    xT:  [D, S] f32r DRAM     (x^T for this batch element)
    wqT/wkT/wvT/woT: [D, D] f32r DRAM  (W.T of the torch-Linear weights)
    bqs: [P, NE] f32 DRAM     (0.125*bq laid out [partition, e-chunk])
    out: [S, D] f32 DRAM      (missing the constant bv@Wo.T+bo row)
    """
    nc = tc.nc
    SC = _chunks(S, P)            # k-chunks, e.g. 11x128 + 92
    QBS = _qblocks(S)
    NSC = len(SC)

    const = ctx.enter_context(tc.tile_pool(name="const", bufs=1))
    qkv = ctx.enter_context(tc.tile_pool(name="qkv", bufs=1))
    gen_ps = ctx.enter_context(tc.tile_pool(name="gen_ps", bufs=2, space="PSUM"))
    sc_ps = ctx.enter_context(tc.tile_pool(name="sc_ps", bufs=2, space="PSUM"))
    ctx_ps = ctx.enter_context(tc.tile_pool(name="ctx_ps", bufs=2, space="PSUM"))
    e_pool = ctx.enter_context(tc.tile_pool(name="epool", bufs=2))
    ctxn_pool = ctx.enter_context(tc.tile_pool(name="ctxn", bufs=3))
    craw_pool = ctx.enter_context(tc.tile_pool(name="craw", bufs=4))
    rb_pool = ctx.enter_context(tc.tile_pool(name="rbp", bufs=4))
    out_sb_pool = ctx.enter_context(tc.tile_pool(name="outsb", bufs=8))
    p3part_pool = ctx.enter_context(tc.tile_pool(name="p3part", bufs=3))
    qt_rd = ctx.enter_context(tc.tile_pool(name="qtrd", bufs=4))
    qt_st = ctx.enter_context(tc.tile_pool(name="qtst", bufs=2))
    dram = ctx.enter_context(tc.tile_pool(name="dram", bufs=8, space="DRAM"))
    dram1 = ctx.enter_context(tc.tile_pool(name="dram1", bufs=1, space="DRAM"))

    # Persistent operands
    kt_pool = ctx.enter_context(tc.tile_pool(name="ktp", bufs=2))
    V = qkv.tile([P, NSC, H * (DH + 1)], CDT)   # per-head 65th ones column
    qt_dram = dram1.tile([D, S], CDT)
    bq_sb = const.tile([P, NE], F32)
    # ScalarE's DMA queue is empty at startup; keeps the sync queue free
    # for the first pair's weight load (the first matmul's stationary
    # operand).
    nc.scalar.dma_start(out=bq_sb[:], in_=bqs)
    woT_sb = const.tile([P, NE, D], CDT)

    # Initialize only the per-head 65th columns to 1.0 (the softmax
    # denominator accumulators); the 64 data columns per head are fully
    # overwritten by the V-projection evictions. Memsetting all of V
    # would occupy DVE for ~10us at startup and stall the first q-block
    # eviction behind it.
    v4 = V[:, :, :].rearrange("p n (h w) -> p n h w", w=DH + 1)
    nc.vector.memset(v4[:, :, :, DH], 1.0)
    ident_sb = const.tile([P, P], CDT)
    nc.sync.dma_start(out=ident_sb[:], in_=ident)

    if hw_loop:
        # Hardware loop: NEFF size is independent of reps, so wall-clock
        # slope vs reps isolates pure device execution time.
        with tc.For_i(0, reps, 1):
            _emit_body(tc, nc, xT, wqT, wkT, wvT, woT, out, S, SC, QBS, NSC,
                       kt_pool, V, qt_dram, bq_sb, woT_sb, ident_sb,
                       gen_ps, sc_ps, ctx_ps, e_pool, ctxn_pool, craw_pool,
                       rb_pool, out_sb_pool, p3part_pool, qt_rd, qt_st, dram)
    else:
        for _rep in range(reps):
            _emit_body(tc, nc, xT, wqT, wkT, wvT, woT, out, S, SC, QBS, NSC,
                       kt_pool, V, qt_dram, bq_sb, woT_sb, ident_sb,
                       gen_ps, sc_ps, ctx_ps, e_pool, ctxn_pool, craw_pool,
                       rb_pool, out_sb_pool, p3part_pool, qt_rd, qt_st, dram)


def _emit_body(tc, nc, xT, wqT, wkT, wvT, woT, out, S, SC, QBS, NSC,
               kt_pool, V, qt_dram, bq_sb, woT_sb, ident_sb,
               gen_ps, sc_ps, ctx_ps, e_pool, ctxn_pool, craw_pool,
               rb_pool, out_sb_pool, p3part_pool, qt_rd, qt_st, dram):

    def phase2_begin(q0, qw, pr, kt_t):
        qt_sb = qt_rd.tile([P, 512], CDT, tag="qt", name=f"qt_{q0}_{pr}")
        nc.sync.dma_start(out=qt_sb[:, 0:qw],
                          in_=qt_dram[pr * P:(pr + 1) * P, q0:q0 + qw])
        return (q0, qw, pr, kt_t, qt_sb, [])

    def phase2_kc(st, kc):
        (q0, qw, pr, kt_t, qt_sb, es) = st
        (k0, kw) = SC[kc]
        sp = sc_ps.tile([P, 1024], F32, tag="sc", name="sp")
        for hi in range(2):
            nc.tensor.matmul(
                sp[:kw, hi * 512:hi * 512 + qw],
                kt_t[hi * DH:(hi + 1) * DH, k0:k0 + kw],
                qt_sb[hi * DH:(hi + 1) * DH, 0:qw],
                start=True, stop=True)
        e_sb = e_pool.tile([P, 1024], CDT, tag=f"e{kc}", name=f"e_sb{kc}")
        if qw == 512:
            nc.scalar.activation(out=e_sb[:kw, :], in_=sp[:kw, :], func=AF.Exp)
        else:
            for hi in range(2):
                nc.scalar.activation(
                    out=e_sb[:kw, hi * 512:hi * 512 + qw],
                    in_=sp[:kw, hi * 512:hi * 512 + qw], func=AF.Exp)
        es.append(e_sb)

    def phase2_ctx(st, cn):
        """probs @ V with e as STATIONARY [k, q<=128] and V as MOVING
        [k, DH+1]: all 128 output partitions used per pass at 65 moving
        rows per (k-chunk, head) -- less than half the PE rows of the
        V-stationary form (512 rows). The softmax denominator lands as a
        per-PARTITION scalar (q is the partition dim), so normalization
        is a plain tensor_scalar -- no partition broadcast at all -- and
        a PE transpose restores the [e, s] layout phase3 consumes."""
        (q0, qw, pr, kt_t, qt_sb, es) = st
        for (qs0, qsw) in _chunks(qw, P):
            for hi in range(2):
                h = 2 * pr + hi
                cps = ctx_ps.tile([P, DH + 1], F32, tag="ctx", name="cps")
                for kc in range(NSC):
                    (k0, kw) = SC[kc]
                    nc.tensor.matmul(
                        cps[:qsw, :],
                        es[kc][:kw, hi * 512 + qs0:hi * 512 + qs0 + qsw],
                        V[:kw, kc, h * (DH + 1):(h + 1) * (DH + 1)],
                        start=(kc == 0), stop=(kc == NSC - 1))
                rcol = craw_pool.tile([P, 1], F32, tag="rc", name="rcol")
                nc.vector.reciprocal(out=rcol[:qsw, :],
                                     in_=cps[:qsw, DH:DH + 1])
                cnT = craw_pool.tile([P, DH], CDT, tag="cnT", name="cnT")
                with nc.allow_low_precision(reason="bf16 normalized ctx"):
                    nc.vector.tensor_scalar_mul(
                        cnT[:qsw, :], cps[:qsw, 0:DH], rcol[:qsw, :])
                tp = gen_ps.tile([DH, P], CDT, tag="mm", name="tp")
                nc.tensor.matmul(tp[:, 0:qsw], cnT[:qsw, 0:DH],
                                 ident_sb[:qsw, 0:qsw], is_transpose=True)
                nc.vector.tensor_copy(
                    out=cn[hi * DH:(hi + 1) * DH, pr, qs0:qs0 + qsw],
                    in_=tp[:, 0:qsw])

    def phase2_scores(q0, qw, pr, kt_t):
        st = phase2_begin(q0, qw, pr, kt_t)
        for kc in range(NSC):
            phase2_kc(st, kc)
        return st

    def phase3(q0, qw, cn, s_sel=None, split_last=False, tail=False):
        schunks = _chunks(qw, P)
        for si, (s0, sw) in enumerate(schunks):
            if s_sel is not None and si not in s_sel:
                continue
            for (o0, ow) in ((0, 512), (512, 256)):
                ot = out_sb_pool.tile([P, 512], CDT, tag="ot", name="ot")
                par = (si * 2 + (o0 > 0)) % 2
                if split_last:
                    # Two-phase accumulation: pairs 0..NE-2 finish and evict
                    # to SBUF without waiting for the LAST pair's
                    # normalization chain; pair NE-1 joins via a single late
                    # matmul folded into the final (add) eviction. Keeps
                    # gen_ps buffers free while the chain is in flight.
                    op_t = gen_ps.tile([P, 512], F32, tag="mm", name="op_t")
                    for ec in range(NE - 1):
                        nc.tensor.matmul(
                            op_t[:sw, :ow],
                            cn[:, ec, s0:s0 + sw],
                            woT_sb[:, ec, o0:o0 + ow],
                            start=(ec == 0), stop=(ec == NE - 2))
                    part = p3part_pool.tile([P, 512], F32, tag="p3p",
                                            name="p3part")
                    nc.vector.tensor_copy(out=part[:sw, :ow],
                                          in_=op_t[:sw, :ow])
                    op5 = gen_ps.tile([P, 512], F32, tag="mm", name="op5")
                    nc.tensor.matmul(
                        op5[:sw, :ow],
                        cn[:, NE - 1, s0:s0 + sw],
                        woT_sb[:, NE - 1, o0:o0 + ow],
                        start=True, stop=True)
                    with nc.allow_low_precision(reason="bf16 output"):
                        nc.vector.tensor_tensor(
                            out=ot[:sw, :ow], in0=part[:sw, :ow],
                            in1=op5[:sw, :ow], op=OP.add)
                else:
                    op_t = gen_ps.tile([P, 512], F32, tag="mm", name="op_t")
                    for ec in range(NE):
                        nc.tensor.matmul(
                            op_t[:sw, :ow],
                            cn[:, ec, s0:s0 + sw],
                            woT_sb[:, ec, o0:o0 + ow],
                            start=(ec == 0), stop=(ec == NE - 1))
                    with nc.allow_low_precision(reason="bf16 output"):
                        if tail and par == 0:
                            # exps are done by the last q-block's output
                            # stage -- ScalarE is idle, so split the
                            # eviction drain across ACT and DVE.
                            nc.scalar.activation(out=ot[:sw, :ow],
                                                 in_=op_t[:sw, :ow],
                                                 func=AF.Copy)
                        else:
                            nc.vector.tensor_copy(out=ot[:sw, :ow],
                                                  in_=op_t[:sw, :ow])
                deng = nc.gpsimd if par == 0 else nc.sync
                deng.dma_start(out=out[q0 + s0:q0 + s0 + sw, o0:o0 + ow],
                               in_=ot[:sw, :ow])

    with tc.tile_pool(name="xw", bufs=1) as xw, \
         tc.tile_pool(name="wecp", bufs=2) as wecp:
        xT_sb = xw.tile([P, ND, S], CDT)
        # Pair 0's q-weights go first on the sync queue so the very first
        # projection matmul has its stationary operand by ~1us; the first
        # q-block's xT chunks alternate across both DMA queues so they
        # arrive at ~2x the single-queue rate.
        pre0 = {"q": wecp.tile([P, ND, P], CDT, tag="wec", name="wec_q0")}
        nc.sync.dma_start(out=pre0["q"][:, :, :], in_=wqT[0])
        for qi, (q0, qw) in enumerate(QBS):
            if qi == 0:
                # fine-grained + dual-queue so the first projection's
                # operands land as early as possible
                for dc in range(ND):
                    eng = nc.sync if dc % 2 == 1 else nc.gpsimd
                    eng.dma_start(
                        out=xT_sb[:, dc, q0:q0 + qw],
                        in_=xT[dc * P:(dc + 1) * P, q0:q0 + qw])
                pre0["k"] = wecp.tile([P, ND, P], CDT, tag="wec",
                                      name="wec_k0")
                nc.sync.dma_start(out=pre0["k"][:, :, :], in_=wkT[0])
            else:
                # one strided DMA per q-block: descriptor generation is
                # ~500ns per dma_start, so 6 small loads would throttle
                # the Pool queue for the rest of the startup
                nc.gpsimd.dma_start(
                    out=xT_sb[:, :, q0:q0 + qw],
                    in_=xT.rearrange("(c p) s -> p c s", p=P)[
                        :, :, q0:q0 + qw])
        # woT loads behind xT on the same queue: first needed by phase3,
        # which only runs near the end of the body.
        nc.gpsimd.dma_start(out=woT_sb[:, :, :],
                            in_=woT.rearrange("(c p) e -> p c e", p=P))

        def emit_kq(ec, pre=None):
            kt_t = kt_pool.tile([P, S], CDT, tag="kt", name=f"kt{ec}")
            for kind, wdram in (("q", wqT), ("k", wkT)):
                if pre is not None and kind in pre:
                    wec = pre[kind]
                else:
                    wec = wecp.tile([P, ND, P], CDT, tag="wec",
                                    name=f"wec_{kind}{ec}")
                    nc.sync.dma_start(out=wec[:, :, :], in_=wdram[ec])
                for (q0, qw) in QBS:
                    ps = gen_ps.tile([P, 512], F32, tag="mm", name="kq_ps")
                    for dc in range(ND):
                        nc.tensor.matmul(
                            ps[:, :qw],
                            wec[:, dc, :],
                            xT_sb[:, dc, q0:q0 + qw],
                            start=(dc == 0), stop=(dc == ND - 1))
                    if kind == "q":
                        qs = qt_st.tile([P, 512], CDT, tag="qs", name="qs")
                        nc.vector.tensor_scalar(
                            out=qs[:, 0:qw], in0=ps[:, :qw],
                            scalar1=SCALE, scalar2=bq_sb[:, ec:ec + 1],
                            op0=OP.mult, op1=OP.add)
                        nc.sync.dma_start(
                            out=qt_dram[ec * P:(ec + 1) * P, q0:q0 + qw],
                            in_=qs[:, 0:qw])
                    else:
                        nc.vector.tensor_copy(out=kt_t[:, q0:q0 + qw],
                                              in_=ps[:, :qw])
            return kt_t

        def emit_v_chunk(w_sb, sc, s0, sw):
            for eh in range(D // EH):
                ps = gen_ps.tile([P, 512], F32, tag="mm", name="v_ps")
                for dc in range(ND):
                    nc.tensor.matmul(
                        ps[:sw, :EH],
                        xT_sb[:, dc, s0:s0 + sw],
                        w_sb[:, dc, eh * EH:(eh + 1) * EH],
                        start=(dc == 0), stop=(dc == ND - 1))
                vh = V[:sw, sc, :].rearrange("p (h w) -> p h w", w=DH + 1)
                nc.vector.tensor_copy(
                    out=vh[:, eh * (EH // DH):(eh + 1) * (EH // DH), 0:DH],
                    in_=ps[:sw, :EH].rearrange("p (h w) -> p h w", w=DH))

        # pr-major emission (emission order IS program order under Tile):
        # each head-pair's K/Q projection is followed by that pair's
        # attention over ALL q-blocks, so the 6 projection units spread
        # across 18 ACT-bound attention units and ScalarE stays the pacer.
        # The V pass interleaves chunk-by-chunk with the very first pair so
        # exp work starts within ~20us of kernel start. Each q-block's
        # output projection is emitted right after its last pair.
        cns = [ctxn_pool.tile([P, NE, 512], CDT, tag="cn", name=f"cn{_q}")
               for _q in range(len(QBS))]
        # Software pipeline: each unit's ctx phase (PE-cheap, DVE-chained)
        # is emitted AFTER the NEXT unit's scores, so the in-order PE
        # stream always hands ACT its 12 exps before grinding the
        # previous unit's ctx/evictions -- otherwise ACT starves ~5us at
        # every q-block boundary.
        pending = []

        def flush_pending():
            while pending:
                st_, cn_ = pending.pop(0)
                phase2_ctx(st_, cn_)

        for pr in range(NE):
            kt_t = emit_kq(pr, pre=pre0 if pr == 0 else None)
            for qi, (q0, qw) in enumerate(QBS):
                if pr == 0 and qi <= 1:
                    # Spread the V projection (23us of PE) over the first
                    # TWO units' k-chunk interleave: packed into one unit
                    # it paces that unit's exps at ~2.3us instead of
                    # ~1.05us and ACT idles ~15us. The deferred-ctx
                    # pipeline flushes ctx(unit0) only after unit1's
                    # scores, so V still completes in time.
                    st = phase2_begin(q0, qw, 0, kt_t)
                    if qi == 0:
                        w_sb = xw.tile([P, ND, D], CDT, tag="w",
                                       name="w_sb")
                        for dc in range(ND):
                            nc.gpsimd.dma_start(
                                out=w_sb[:, dc, :],
                                in_=wvT[dc * P:(dc + 1) * P, :])
                    half_v = (NSC + 1) // 2
                    for kc in range(NSC):
                        if kc % 2 == 0:
                            sc = qi * half_v + kc // 2
                            if sc < NSC:
                                (s0, sw) = SC[sc]
                                emit_v_chunk(w_sb, sc, s0, sw)
                        phase2_kc(st, kc)
                else:
                    st = phase2_scores(q0, qw, pr, kt_t)
                pending.append((st, cns[qi]))
                if pr == NE - 1:
                    flush_pending()
                    # Interleave: finish the PREVIOUS q-block's second half
                    # here so its PE work covers this block's normalization
                    # chain; the current block keeps its first half only.
                    nsc_q = len(_chunks(qw, P))
                    half = nsc_q // 2
                    if qi > 0:
                        pq0, pqw = QBS[qi - 1]
                        pn = len(_chunks(pqw, P))
                        phase3(pq0, pqw, cns[qi - 1],
                               s_sel=set(range(pn // 2, pn)))
                    if qi < len(QBS) - 1:
                        phase3(q0, qw, cns[qi], s_sel=set(range(half)))
                    else:
                        phase3(q0, qw, cns[qi], tail=True)
                elif len(pending) > 1:
                    phase2_ctx(*pending.pop(0))


def build_nc(S=S_FULL, reps=1, hw_loop=False):
    nc = bacc.Bacc("TRN2", target_bir_lowering=False, debug=False,
                   enable_asserts=False, num_devices=1)
    xT = nc.dram_tensor("xT", [D, S], CDT, kind="ExternalInput").ap()
    wqT = nc.dram_tensor("wqT", [NE, P, D], CDT, kind="ExternalInput").ap()
    wkT = nc.dram_tensor("wkT", [NE, P, D], CDT, kind="ExternalInput").ap()
    wvT = nc.dram_tensor("wvT", [D, D], CDT, kind="ExternalInput").ap()
    woT = nc.dram_tensor("woT", [D, D], CDT, kind="ExternalInput").ap()
    bqs = nc.dram_tensor("bqs", [P, NE], F32, kind="ExternalInput").ap()
    ident = nc.dram_tensor("ident", [P, P], CDT, kind="ExternalInput").ap()
    out = nc.dram_tensor("out", [S, D], CDT, kind="ExternalOutput").ap()
    with tile.TileContext(nc) as tc:
        with ExitStack() as ctx:
            build_attention(tc, ctx, xT, wqT, wkT, wvT, woT, bqs, ident,
                            out, S, reps, hw_loop)
    nc.compile()
    return nc


_NC_CACHE = {}


def _get_nc(S=S_FULL, reps=1, hw_loop=False):
    key = (S, reps, hw_loop)
    if key not in _NC_CACHE:
        _NC_CACHE[key] = build_nc(S, reps, hw_loop)
    return _NC_CACHE[key]


def prep_inputs(x, Wq, bq, Wk, Wv, bv, Wo, bo):
    x = np.asarray(x, dtype=np.float32)
    Wq = np.asarray(Wq, dtype=np.float32)
    Wk = np.asarray(Wk, dtype=np.float32)
    Wv = np.asarray(Wv, dtype=np.float32)
    Wo = np.asarray(Wo, dtype=np.float32)
    bq = np.asarray(bq, dtype=np.float32)
    bv = np.asarray(bv, dtype=np.float32)
    bo = np.asarray(bo, dtype=np.float32)
    xT = np.ascontiguousarray(x.transpose(0, 2, 1)).astype(NPCDT)
    def _sw(wt):
        # [D, D] -> [NE, P, ND*P]: block-column ec, gathered over dc rows
        return np.ascontiguousarray(
            wt.reshape(ND, P, NE, P).transpose(2, 1, 0, 3).reshape(NE, P, D))
    base = {
        "wqT": _sw(Wq.T).astype(NPCDT),
        "wkT": _sw(Wk.T).astype(NPCDT),
        "wvT": np.ascontiguousarray(Wv.T).astype(NPCDT),
        "woT": np.ascontiguousarray(Wo.T).astype(NPCDT),
        "bqs": np.ascontiguousarray((SCALE * bq).reshape(NE, P).T),
        "ident": np.eye(P, dtype=np.float32).astype(NPCDT),
    }
    const_row = (bv @ Wo.T + bo).astype(np.float32)
    in_maps = [dict(base, xT=np.ascontiguousarray(xT[b])) for b in range(x.shape[0])]
    return in_maps, const_row


def kernel(x, Wq, bq, Wk, Wv, bv, Wo, bo):
    in_maps, const_row = prep_inputs(x, Wq, bq, Wk, Wv, bv, Wo, bo)
    nc = _get_nc(x.shape[1])
    res = bass_utils.run_bass_kernel_spmd(
        nc, in_maps, core_ids=list(range(len(in_maps))))
    out = np.stack([np.asarray(r["out"], dtype=np.float32)
                    for r in res.results])
    return (out + const_row[None, None, :]).astype(np.float32)



# revision 61
# speedup vs baseline: 1.1572x; 1.1572x over previous
"""Trainium2 Bass kernel for batched multi-head self-attention.

Problem: x [8, 1500, 768], 12 heads x 64 dims, torch-Linear style projections.
Strategy: data-parallel over batch (1 element per NeuronCore, 8 cores).

Per-core design (host pre-transposes everything; device does no transposes):
  - xT [768, 1500]: projections contract over d on the partition axis.
  - Q^T, K^T in [e, s] layout (pairs of heads per 128-partition chunk).
    K^T and V stay SBUF-resident; Q^T roundtrips through a DRAM scratch
    (each [head-pair, q-block] slice is consumed exactly once).
  - q-blocks are uniformly 512 wide; the last block overlaps the previous
    one (start S-512) so no padding or edge cases exist for S >= 512.
  - scores computed TRANSPOSED: scoresT[k, q] = K_h^T.T @ Q_h^T, two heads
    per PE pass via row tile_position packing (contraction is dh=64 only).
  - exp on ScalarE straight out of PSUM ([128,1024] two-bank spans), no max
    subtraction (scores ~ N(0,1): fp32-safe).
  - softmax denominators ride as a 65th all-ones column of V inside the ctx
    matmul (ctxT psum = 64 ctx rows + 1 sums row).
  - normalization: reciprocal of the sums row, partition-broadcast via a
    tiny DRAM roundtrip, multiplied in during the ctx psum eviction.
  - output projection consumes ctx_normT [e, s] directly; the bv/bo
    contribution is a constant row (softmax rows sum to 1) added on host.

All matmul operands are bfloat16 (1 cycle/row on the PE at any moving
size, half the DMA bytes / SBUF of f32r). PSUM accumulation stays fp32;
softmax denominators come from the same bf16 e values as the numerators,
so normalization is consistent. End-to-end rel err ~5.7e-3 vs fp32
(gate: 2e-2). Output is written bf16 and upcast on host.

Scheduling notes (sim-guided, CoreSim cost model; body ~302us sim):
  - only the 12 per-head ones-columns of V are memset (a full-V memset
    occupies DVE ~10us at startup and stalls the first q-block eviction);
  - Wq/Wk ship host-swizzled as [NE, 128, D] so each per-pair weight
    load is one contiguous DMA (column-block loads of a [D, D] matrix
    pay a 2x sub-512B-row DMA penalty); pair 0's weights preload ahead
    of the xT stream, whose first q-block alternates both DMA queues;
    later q-blocks load as single strided DMAs (descriptor generation
    costs ~500ns per dma_start per queue);
  - woT loads after xT on the same queue (first needed only by phase3);
  - normalization: reciprocal straight off the PSUM sums row (bf16),
    DRAM broadcast roundtrip + Pool-engine multiply in steady state
    (GpSimd cannot touch PSUM, so eviction copies stay on DVE); the
    body-tail unit instead broadcasts via a rank-1 PE matmul
    (ones[1,DH]^T @ rc -> PSUM) to dodge the shared-DMA-engine backlog,
    evicts ctx on the post-exp-idle ScalarE, and multiplies on DVE;
  - output evictions alternate DMA queues (gpsimd/sync); the tail
    q-block also alternates its eviction copies across ScalarE/DVE.
"""

import numpy as np
from contextlib import ExitStack

import concourse.bass as bass
import concourse.bacc as bacc
import concourse.tile as tile
from concourse import mybir
from concourse import bass_utils

F32 = mybir.dt.float32
F32R = mybir.dt.float32r
BF16 = mybir.dt.bfloat16
CDT = BF16               # compute dtype for matmul operands (1 cyc/row)
NPCDT = mybir.dt.np(CDT)
AF = mybir.ActivationFunctionType
OP = mybir.AluOpType

P = 128
D = 768
H = 12
DH = 64
NE = D // P          # 6 e-chunks (head pairs)
ND = D // P          # 6 d-chunks
SCALE = 0.125
S_FULL = 1500
QB = 512
EH = 384             # half of D for the V projection moving dim


def _chunks(total, size):
    out = []
    o = 0
    while o < total:
        out.append((o, min(size, total - o)))
        o += size
    return out


def _qblocks(S):
    """512-wide q-blocks; the last one is narrower (phase 2 handles qw<512
    with split exp instructions). An overlapped uniform-512 last block was
    tried (-9us ACT from single exps) but created +14us of pipeline idle
    -- net loss, reverted."""
    return _chunks(S, QB)


def build_attention(tc, ctx, xT, wqT, wkT, wvT, woT, bqs, ident, out, S,
                    reps=1,
                    hw_loop=False):
    """Emit the single-core attention program.

    xT:  [D, S] f32r DRAM     (x^T for this batch element)
    wqT/wkT/wvT/woT: [D, D] f32r DRAM  (W.T of the torch-Linear weights)
    bqs: [P, NE] f32 DRAM     (0.125*bq laid out [partition, e-chunk])
    out: [S, D] f32 DRAM      (missing the constant bv@Wo.T+bo row)
    """
    nc = tc.nc
    SC = _chunks(S, P)            # k-chunks, e.g. 11x128 + 92
    QBS = _qblocks(S)
    NSC = len(SC)

    const = ctx.enter_context(tc.tile_pool(name="const", bufs=1))
    qkv = ctx.enter_context(tc.tile_pool(name="qkv", bufs=1))
    gen_ps = ctx.enter_context(tc.tile_pool(name="gen_ps", bufs=2, space="PSUM"))
    sc_ps = ctx.enter_context(tc.tile_pool(name="sc_ps", bufs=2, space="PSUM"))
    ctx_ps = ctx.enter_context(tc.tile_pool(name="ctx_ps", bufs=2, space="PSUM"))
    e_pool = ctx.enter_context(tc.tile_pool(name="epool", bufs=2))
    ctxn_pool = ctx.enter_context(tc.tile_pool(name="ctxn", bufs=3))
    craw_pool = ctx.enter_context(tc.tile_pool(name="craw", bufs=4))
    rb_pool = ctx.enter_context(tc.tile_pool(name="rbp", bufs=4))
    out_sb_pool = ctx.enter_context(tc.tile_pool(name="outsb", bufs=8))
    p3part_pool = ctx.enter_context(tc.tile_pool(name="p3part", bufs=3))
    qt_rd = ctx.enter_context(tc.tile_pool(name="qtrd", bufs=4))
    qt_st = ctx.enter_context(tc.tile_pool(name="qtst", bufs=2))
    dram = ctx.enter_context(tc.tile_pool(name="dram", bufs=8, space="DRAM"))
    dram1 = ctx.enter_context(tc.tile_pool(name="dram1", bufs=1, space="DRAM"))

    # Persistent operands
    kt_pool = ctx.enter_context(tc.tile_pool(name="ktp", bufs=2))
    V = qkv.tile([P, NSC, H * (DH + 1)], CDT)   # per-head 65th ones column
    qt_dram = dram1.tile([D, S], CDT)
    bq_sb = const.tile([P, NE], F32)
    # ScalarE's DMA queue is empty at startup; keeps the sync queue free
    # for the first pair's weight load (the first matmul's stationary
    # operand).
    nc.scalar.dma_start(out=bq_sb[:], in_=bqs)
    woT_sb = const.tile([P, NE, D], CDT)

    # Initialize only the per-head 65th columns to 1.0 (the softmax
    # denominator accumulators); the 64 data columns per head are fully
    # overwritten by the V-projection evictions. Memsetting all of V
    # would occupy DVE for ~10us at startup and stall the first q-block
    # eviction behind it.
    v4 = V[:, :, :].rearrange("p n (h w) -> p n h w", w=DH + 1)
    nc.vector.memset(v4[:, :, :, DH], 1.0)
    ident_sb = const.tile([P, P], CDT)
    nc.sync.dma_start(out=ident_sb[:], in_=ident)

    if hw_loop:
        # Hardware loop: NEFF size is independent of reps, so wall-clock
        # slope vs reps isolates pure device execution time.
        with tc.For_i(0, reps, 1):
            _emit_body(tc, nc, xT, wqT, wkT, wvT, woT, out, S, SC, QBS, NSC,
                       kt_pool, V, qt_dram, bq_sb, woT_sb, ident_sb,
                       gen_ps, sc_ps, ctx_ps, e_pool, ctxn_pool, craw_pool,
                       rb_pool, out_sb_pool, p3part_pool, qt_rd, qt_st, dram)
    else:
        for _rep in range(reps):
            _emit_body(tc, nc, xT, wqT, wkT, wvT, woT, out, S, SC, QBS, NSC,
                       kt_pool, V, qt_dram, bq_sb, woT_sb, ident_sb,
                       gen_ps, sc_ps, ctx_ps, e_pool, ctxn_pool, craw_pool,
                       rb_pool, out_sb_pool, p3part_pool, qt_rd, qt_st, dram)


def _emit_body(tc, nc, xT, wqT, wkT, wvT, woT, out, S, SC, QBS, NSC,
               kt_pool, V, qt_dram, bq_sb, woT_sb, ident_sb,
               gen_ps, sc_ps, ctx_ps, e_pool, ctxn_pool, craw_pool,
               rb_pool, out_sb_pool, p3part_pool, qt_rd, qt_st, dram):

    def phase2_begin(q0, qw, pr, kt_t, qts):
        return (q0, qw, pr, kt_t, qts[q0], [])

    def phase2_kc(st, kc):
        (q0, qw, pr, kt_t, qt_sb, es) = st
        (k0, kw) = SC[kc]
        sp = sc_ps.tile([P, 1024], F32, tag="sc", name="sp")
        # hi*512 offsets: a matmul output may not cross the 512-f32 PSUM
        # bank boundary, so the narrow last q-block cannot pack the two
        # heads contiguously and pays a split exp instead.
        for hi in range(2):
            nc.tensor.matmul(
                sp[:kw, hi * 512:hi * 512 + qw],
                kt_t[hi * DH:(hi + 1) * DH, k0:k0 + kw],
                qt_sb[hi * DH:(hi + 1) * DH, 0:qw],
                start=True, stop=True)
        e_sb = e_pool.tile([P, 1024], CDT, tag=f"e{kc}", name=f"e_sb{kc}")
        # NOTE: a single exp spanning the [qw, 512) gap between the heads
        # for narrow blocks is ILLEGAL -- the gap belongs to another tile
        # generation, so the ACT read races a concurrent score matmul's
        # PE write to the same bank region (fatal hazard class).
        if qw == 512:
            nc.scalar.activation(out=e_sb[:kw, :], in_=sp[:kw, :],
                                 func=AF.Exp)
        else:
            for hi in range(2):
                nc.scalar.activation(
                    out=e_sb[:kw, hi * 512:hi * 512 + qw],
                    in_=sp[:kw, hi * 512:hi * 512 + qw], func=AF.Exp)
        es.append(e_sb)

    def phase2_ctx(st, cn):
        """probs @ V with e as STATIONARY [k, q<=128] and V as MOVING
        [k, DH+1]: all 128 output partitions used per pass at 65 moving
        rows per (k-chunk, head) -- less than half the PE rows of the
        V-stationary form (512 rows). The softmax denominator lands as a
        per-PARTITION scalar (q is the partition dim), so normalization
        is a plain tensor_scalar -- no partition broadcast at all -- and
        a PE transpose restores the [e, s] layout phase3 consumes."""
        (q0, qw, pr, kt_t, qt_sb, es) = st
        for (qs0, qsw) in _chunks(qw, P):
            cnT = craw_pool.tile([P, 2 * DH], CDT, tag="cnT", name="cnT")
            for hi in range(2):
                h = 2 * pr + hi
                cps = ctx_ps.tile([P, DH + 1], F32, tag="ctx", name="cps")
                for kc in range(NSC):
                    (k0, kw) = SC[kc]
                    nc.tensor.matmul(
                        cps[:qsw, :],
                        es[kc][:kw, hi * 512 + qs0:hi * 512 + qs0 + qsw],
                        V[:kw, kc, h * (DH + 1):(h + 1) * (DH + 1)],
                        start=(kc == 0), stop=(kc == NSC - 1))
                rcol = craw_pool.tile([P, 1], F32, tag="rc", name="rcol")
                nc.vector.reciprocal(out=rcol[:qsw, :],
                                     in_=cps[:qsw, DH:DH + 1])
                with nc.allow_low_precision(reason="bf16 normalized ctx"):
                    nc.vector.tensor_scalar_mul(
                        cnT[:qsw, hi * DH:(hi + 1) * DH],
                        cps[:qsw, 0:DH], rcol[:qsw, :])
            # Both heads' normalized tiles share one [qsw, 128] SBUF tile,
            # so a SINGLE transpose+eviction restores the full [e, s]
            # slice -- half the transpose rows of per-head transposes.
            tp = gen_ps.tile([P, P], CDT, tag="mm", name="tp")
            nc.tensor.matmul(tp[:, 0:qsw], cnT[:qsw, 0:2 * DH],
                             ident_sb[:qsw, 0:qsw], is_transpose=True)
            nc.vector.tensor_copy(
                out=cn[:, pr, qs0:qs0 + qsw],
                in_=tp[:, 0:qsw])

    def phase2_scores(q0, qw, pr, kt_t, qts):
        st = phase2_begin(q0, qw, pr, kt_t, qts)
        for kc in range(NSC):
            phase2_kc(st, kc)
        return st

    def phase3_partials(q0, qw, cn, s_sel=None, s_off=0):
        # Pairs 0..NE-2 of the output projection for this q-block: their
        # ctx slices were written during those pairs' own iterations, so
        # these accumulations are ready LONG before the last pair -- emit
        # them early (priority-wise) so they soak up mid-stream PE gaps
        # instead of serializing in the tail.
        parts = {}
        schunks = _chunks(qw, P)
        if s_off:
            schunks[0] = (s_off, schunks[0][1] - s_off)
        for si, (s0, sw) in enumerate(schunks):
            if s_sel is not None and si not in s_sel:
                continue
            for (o0, ow) in ((0, 512), (512, 256)):
                op_t = gen_ps.tile([P, 512], F32, tag="mm", name="op_t")
                for ec in range(NE - 2):
                    nc.tensor.matmul(
                        op_t[:sw, :ow],
                        cn[:, ec, s0:s0 + sw],
                        woT_sb[:, ec, o0:o0 + ow],
                        start=(ec == 0), stop=(ec == NE - 3))
                part = p3part_pool.tile([P, 512], F32, tag=f"p3p{si}{o0}",
                                        name="p3part")
                nc.vector.tensor_copy(out=part[:sw, :ow],
                                      in_=op_t[:sw, :ow])
                parts[(si, o0)] = part
        return parts

    def phase3_final(q0, qw, cn, parts, s_sel=None, s_off=0):
        # Only the last pair's single accumulation + the add-eviction
        # remain on the critical tail.
        schunks = _chunks(qw, P)
        if s_off:
            schunks[0] = (s_off, schunks[0][1] - s_off)
        for si, (s0, sw) in enumerate(schunks):
            if s_sel is not None and si not in s_sel:
                continue
            for (o0, ow) in ((0, 512), (512, 256)):
                op5 = gen_ps.tile([P, 512], F32, tag="mm", name="op5")
                for ec in (NE - 2, NE - 1):
                    nc.tensor.matmul(
                        op5[:sw, :ow],
                        cn[:, ec, s0:s0 + sw],
                        woT_sb[:, ec, o0:o0 + ow],
                        start=(ec == NE - 2), stop=(ec == NE - 1))
                ot = out_sb_pool.tile([P, 512], CDT, tag="ot", name="ot")
                with nc.allow_low_precision(reason="bf16 output"):
                    nc.vector.tensor_tensor(
                        out=ot[:sw, :ow], in0=parts[(si, o0)][:sw, :ow],
                        in1=op5[:sw, :ow], op=OP.add)
                par = (si * 2 + (o0 > 0)) % 2
                deng = nc.gpsimd if par == 0 else nc.sync
                deng.dma_start(out=out[q0 + s0:q0 + s0 + sw, o0:o0 + ow],
                               in_=ot[:sw, :ow])

    def phase3(q0, qw, cn, s_sel=None, split_last=False, tail=False):
        schunks = _chunks(qw, P)
        for si, (s0, sw) in enumerate(schunks):
            if s_sel is not None and si not in s_sel:
                continue
            for (o0, ow) in ((0, 512), (512, 256)):
                ot = out_sb_pool.tile([P, 512], CDT, tag="ot", name="ot")
                par = (si * 2 + (o0 > 0)) % 2
                if split_last:
                    # Two-phase accumulation: pairs 0..NE-2 finish and evict
                    # to SBUF without waiting for the LAST pair's
                    # normalization chain; pair NE-1 joins via a single late
                    # matmul folded into the final (add) eviction. Keeps
                    # gen_ps buffers free while the chain is in flight.
                    op_t = gen_ps.tile([P, 512], F32, tag="mm", name="op_t")
                    for ec in range(NE - 1):
                        nc.tensor.matmul(
                            op_t[:sw, :ow],
                            cn[:, ec, s0:s0 + sw],
                            woT_sb[:, ec, o0:o0 + ow],
                            start=(ec == 0), stop=(ec == NE - 2))
                    part = p3part_pool.tile([P, 512], F32, tag="p3p",
                                            name="p3part")
                    nc.vector.tensor_copy(out=part[:sw, :ow],
                                          in_=op_t[:sw, :ow])
                    op5 = gen_ps.tile([P, 512], F32, tag="mm", name="op5")
                    nc.tensor.matmul(
                        op5[:sw, :ow],
                        cn[:, NE - 1, s0:s0 + sw],
                        woT_sb[:, NE - 1, o0:o0 + ow],
                        start=True, stop=True)
                    with nc.allow_low_precision(reason="bf16 output"):
                        nc.vector.tensor_tensor(
                            out=ot[:sw, :ow], in0=part[:sw, :ow],
                            in1=op5[:sw, :ow], op=OP.add)
                else:
                    op_t = gen_ps.tile([P, 512], F32, tag="mm", name="op_t")
                    for ec in range(NE):
                        nc.tensor.matmul(
                            op_t[:sw, :ow],
                            cn[:, ec, s0:s0 + sw],
                            woT_sb[:, ec, o0:o0 + ow],
                            start=(ec == 0), stop=(ec == NE - 1))
                    with nc.allow_low_precision(reason="bf16 output"):
                        nc.vector.tensor_copy(out=ot[:sw, :ow],
                                              in_=op_t[:sw, :ow])
                deng = nc.gpsimd if par == 0 else nc.sync
                deng.dma_start(out=out[q0 + s0:q0 + s0 + sw, o0:o0 + ow],
                               in_=ot[:sw, :ow])

    with tc.tile_pool(name="xw", bufs=1) as xw, \
         tc.tile_pool(name="wecp", bufs=2) as wecp:
        xT_sb = xw.tile([P, ND, S], CDT)
        # Pair 0's q-weights go first on the sync queue so the very first
        # projection matmul has its stationary operand by ~1us; the first
        # q-block's xT chunks alternate across both DMA queues so they
        # arrive at ~2x the single-queue rate.
        pre0 = {"q": wecp.tile([P, ND, P], CDT, tag="wec", name="wec_q0")}
        nc.sync.dma_start(out=pre0["q"][:, :, :], in_=wqT[0])
        for qi, (q0, qw) in enumerate(QBS):
            if qi == 0:
                # fine-grained + dual-queue so the first projection's
                # operands land as early as possible
                for dc in range(ND):
                    eng = nc.sync if dc % 2 == 1 else nc.gpsimd
                    eng.dma_start(
                        out=xT_sb[:, dc, q0:q0 + qw],
                        in_=xT[dc * P:(dc + 1) * P, q0:q0 + qw])
                pre0["k"] = wecp.tile([P, ND, P], CDT, tag="wec",
                                      name="wec_k0")
                nc.sync.dma_start(out=pre0["k"][:, :, :], in_=wkT[0])
            else:
                # one strided DMA per q-block: descriptor generation is
                # ~500ns per dma_start, so 6 small loads would throttle
                # the Pool queue for the rest of the startup
                nc.gpsimd.dma_start(
                    out=xT_sb[:, :, q0:q0 + qw],
                    in_=xT.rearrange("(c p) s -> p c s", p=P)[
                        :, :, q0:q0 + qw])
        # woT loads behind xT on the same queue: first needed by phase3,
        # which only runs near the end of the body.
        nc.gpsimd.dma_start(out=woT_sb[:, :, :],
                            in_=woT.rearrange("(c p) e -> p c e", p=P))

        def emit_kq(ec, pre=None):
            kt_t = kt_pool.tile([P, S], CDT, tag="kt", name=f"kt{ec}")
            qts = {}
            wecs = {}
            for kind, wdram in (("q", wqT), ("k", wkT)):
                if pre is not None and kind in pre:
                    wecs[kind] = pre[kind]
                else:
                    wecs[kind] = wecp.tile([P, ND, P], CDT, tag="wec",
                                           name=f"wec_{kind}{ec}")
                    nc.sync.dma_start(out=wecs[kind][:, :, :], in_=wdram[ec])
            # q-block-major: the first scores need q AND k of q-block 0,
            # so emit q(qb0), k(qb0) before the later q-blocks instead of
            # all of q before all of k. K uses NON-overlapping blocks
            # (an overlapped last block would double-write kt columns and
            # serialize every unit's scores on that eviction); only Q
            # needs the uniform-512 overlap for the single-exp win.
            kblocks = _chunks(S, QB)
            for bi in range(len(QBS)):
                for kind in ("q", "k"):
                    (q0, qw) = QBS[bi] if kind == "q" else kblocks[bi]
                    wec = wecs[kind]
                    ps = gen_ps.tile([P, 512], F32, tag="mm", name="kq_ps")
                    for dc in range(ND):
                        nc.tensor.matmul(
                            ps[:, :qw],
                            wec[:, dc, :],
                            xT_sb[:, dc, q0:q0 + qw],
                            start=(dc == 0), stop=(dc == ND - 1))
                    if kind == "q":
                        # Each pair's Q is consumed only by this pair's own
                        # phase2, which follows immediately -- keep it
                        # SBUF-resident instead of the legacy DRAM
                        # roundtrip (36 DMAs + latency on the first score).
                        qs = qt_st.tile([P, 512], CDT, tag=f"qs{q0}",
                                        name=f"qs_{ec}_{q0}")
                        nc.vector.tensor_scalar(
                            out=qs[:, 0:qw], in0=ps[:, :qw],
                            scalar1=SCALE, scalar2=bq_sb[:, ec:ec + 1],
                            op0=OP.mult, op1=OP.add)
                        qts[q0] = qs
                    else:
                        nc.vector.tensor_copy(out=kt_t[:, q0:q0 + qw],
                                              in_=ps[:, :qw])
            return kt_t, qts

        def emit_v_chunk(w_sb, sc, s0, sw):
            for eh in range(D // EH):
                ps = gen_ps.tile([P, 512], F32, tag="mm", name="v_ps")
                for dc in range(ND):
                    nc.tensor.matmul(
                        ps[:sw, :EH],
                        xT_sb[:, dc, s0:s0 + sw],
                        w_sb[:, dc, eh * EH:(eh + 1) * EH],
                        start=(dc == 0), stop=(dc == ND - 1))
                vh = V[:sw, sc, :].rearrange("p (h w) -> p h w", w=DH + 1)
                nc.vector.tensor_copy(
                    out=vh[:, eh * (EH // DH):(eh + 1) * (EH // DH), 0:DH],
                    in_=ps[:sw, :EH].rearrange("p (h w) -> p h w", w=DH))

        # pr-major emission (emission order IS program order under Tile):
        # each head-pair's K/Q projection is followed by that pair's
        # attention over ALL q-blocks, so the 6 projection units spread
        # across 18 ACT-bound attention units and ScalarE stays the pacer.
        # The V pass interleaves chunk-by-chunk with the very first pair so
        # exp work starts within ~20us of kernel start. Each q-block's
        # output projection is emitted right after its last pair.
        cns = [ctxn_pool.tile([P, NE, 512], CDT, tag="cn", name=f"cn{_q}")
               for _q in range(len(QBS))]
        # Software pipeline: each unit's ctx phase (PE-cheap, DVE-chained)
        # is emitted AFTER the NEXT unit's scores, so the in-order PE
        # stream always hands ACT its 12 exps before grinding the
        # previous unit's ctx/evictions -- otherwise ACT starves ~5us at
        # every q-block boundary.
        pending = []

        def flush_pending():
            while pending:
                st_, cn_ = pending.pop(0)
                phase2_ctx(st_, cn_)

        for pr in range(NE):
            kt_t, qts = emit_kq(pr, pre=pre0 if pr == 0 else None)
            for qi, (q0, qw) in enumerate(QBS):
                if pr == 0 and qi <= 1:
                    # Spread the V projection (23us of PE) over the first
                    # TWO units' k-chunk interleave: packed into one unit
                    # it paces that unit's exps at ~2.3us instead of
                    # ~1.05us and ACT idles ~15us. The deferred-ctx
                    # pipeline flushes ctx(unit0) only after unit1's
                    # scores, so V still completes in time.
                    st = phase2_begin(q0, qw, 0, kt_t, qts)
                    if qi == 0:
                        w_sb = xw.tile([P, ND, D], CDT, tag="w",
                                       name="w_sb")
                        for dc in range(ND):
                            nc.gpsimd.dma_start(
                                out=w_sb[:, dc, :],
                                in_=wvT[dc * P:(dc + 1) * P, :])
                    half_v = (NSC + 1) // 2
                    for kc in range(NSC):
                        if kc % 2 == 0:
                            sc = qi * half_v + kc // 2
                            if sc < NSC:
                                (s0, sw) = SC[sc]
                                emit_v_chunk(w_sb, sc, s0, sw)
                        phase2_kc(st, kc)
                else:
                    st = phase2_scores(q0, qw, pr, kt_t, qts)
                pending.append((st, cns[qi]))
                ql0, qlw = QBS[-1]
                ov = (QBS[-2][0] + QBS[-2][1]) - ql0 if len(QBS) > 1 else 0
                if pr == NE - 2 and qi == 0:
                    # pairs 0..NE-3 have finished the last q-block's ctx by
                    # now (the deferred pipeline is one unit behind): emit
                    # the partial output accumulations here so they fill
                    # PE gaps across the last TWO pairs' stretches.
                    flush_pending()
                    q2parts = phase3_partials(ql0, qlw, cns[-1], s_off=ov)
                if pr == NE - 1:
                    flush_pending()
                    # Interleave: finish the PREVIOUS q-block's second half
                    # here so its PE work covers this block's normalization
                    # chain; the current block keeps its first half only.
                    nsc_q = len(_chunks(qw, P))
                    half = nsc_q // 2
                    if qi > 0:
                        pq0, pqw = QBS[qi - 1]
                        pn = len(_chunks(pqw, P))
                        phase3(pq0, pqw, cns[qi - 1],
                               s_sel=set(range(pn // 2, pn)))
                    if qi < len(QBS) - 1:
                        phase3(q0, qw, cns[qi], s_sel=set(range(half)))
                    else:
                        phase3_final(q0, qw, cns[qi], q2parts, s_off=ov)
                elif len(pending) > 1:
                    phase2_ctx(*pending.pop(0))


def build_nc(S=S_FULL, reps=1, hw_loop=False):
    nc = bacc.Bacc("TRN2", target_bir_lowering=False, debug=False,
                   enable_asserts=False, num_devices=1)
    xT = nc.dram_tensor("xT", [D, S], CDT, kind="ExternalInput").ap()
    wqT = nc.dram_tensor("wqT", [NE, P, D], CDT, kind="ExternalInput").ap()
    wkT = nc.dram_tensor("wkT", [NE, P, D], CDT, kind="ExternalInput").ap()
    wvT = nc.dram_tensor("wvT", [D, D], CDT, kind="ExternalInput").ap()
    woT = nc.dram_tensor("woT", [D, D], CDT, kind="ExternalInput").ap()
    bqs = nc.dram_tensor("bqs", [P, NE], F32, kind="ExternalInput").ap()
    ident = nc.dram_tensor("ident", [P, P], CDT, kind="ExternalInput").ap()
    out = nc.dram_tensor("out", [S, D], CDT, kind="ExternalOutput").ap()
    with tile.TileContext(nc) as tc:
        with ExitStack() as ctx:
            build_attention(tc, ctx, xT, wqT, wkT, wvT, woT, bqs, ident,
                            out, S, reps, hw_loop)
    nc.compile()
    return nc


_NC_CACHE = {}


def _get_nc(S=S_FULL, reps=1, hw_loop=False):
    key = (S, reps, hw_loop)
    if key not in _NC_CACHE:
        _NC_CACHE[key] = build_nc(S, reps, hw_loop)
    return _NC_CACHE[key]


def prep_inputs(x, Wq, bq, Wk, Wv, bv, Wo, bo):
    x = np.asarray(x, dtype=np.float32)
    Wq = np.asarray(Wq, dtype=np.float32)
    Wk = np.asarray(Wk, dtype=np.float32)
    Wv = np.asarray(Wv, dtype=np.float32)
    Wo = np.asarray(Wo, dtype=np.float32)
    bq = np.asarray(bq, dtype=np.float32)
    bv = np.asarray(bv, dtype=np.float32)
    bo = np.asarray(bo, dtype=np.float32)
    xT = np.ascontiguousarray(x.transpose(0, 2, 1)).astype(NPCDT)
    def _sw(wt):
        # [D, D] -> [NE, P, ND*P]: block-column ec, gathered over dc rows
        return np.ascontiguousarray(
            wt.reshape(ND, P, NE, P).transpose(2, 1, 0, 3).reshape(NE, P, D))
    base = {
        "wqT": _sw(Wq.T).astype(NPCDT),
        "wkT": _sw(Wk.T).astype(NPCDT),
        "wvT": np.ascontiguousarray(Wv.T).astype(NPCDT),
        "woT": np.ascontiguousarray(Wo.T).astype(NPCDT),
        "bqs": np.ascontiguousarray((SCALE * bq).reshape(NE, P).T),
        "ident": np.eye(P, dtype=np.float32).astype(NPCDT),
    }
    const_row = (bv @ Wo.T + bo).astype(np.float32)
    in_maps = [dict(base, xT=np.ascontiguousarray(xT[b])) for b in range(x.shape[0])]
    return in_maps, const_row


def kernel(x, Wq, bq, Wk, Wv, bv, Wo, bo):
    in_maps, const_row = prep_inputs(x, Wq, bq, Wk, Wv, bv, Wo, bo)
    nc = _get_nc(x.shape[1])
    res = bass_utils.run_bass_kernel_spmd(
        nc, in_maps, core_ids=list(range(len(in_maps))))
    out = np.stack([np.asarray(r["out"], dtype=np.float32)
                    for r in res.results])
    return (out + const_row[None, None, :]).astype(np.float32)



# revision 66
# speedup vs baseline: 1.1592x; 1.0018x over previous
"""Trainium2 Bass kernel for batched multi-head self-attention.

Problem: x [8, 1500, 768], 12 heads x 64 dims, torch-Linear style projections.
Strategy: data-parallel over batch (1 element per NeuronCore, 8 cores).

Per-core design (host pre-transposes everything; device does no transposes):
  - xT [768, 1500]: projections contract over d on the partition axis.
  - Q^T, K^T in [e, s] layout (pairs of heads per 128-partition chunk).
    K^T and V stay SBUF-resident; Q^T roundtrips through a DRAM scratch
    (each [head-pair, q-block] slice is consumed exactly once).
  - q-blocks are uniformly 512 wide; the last block overlaps the previous
    one (start S-512) so no padding or edge cases exist for S >= 512.
  - scores computed TRANSPOSED: scoresT[k, q] = K_h^T.T @ Q_h^T, two heads
    per PE pass via row tile_position packing (contraction is dh=64 only).
  - exp on ScalarE straight out of PSUM ([128,1024] two-bank spans), no max
    subtraction (scores ~ N(0,1): fp32-safe).
  - softmax denominators ride as a 65th all-ones column of V inside the ctx
    matmul (ctxT psum = 64 ctx rows + 1 sums row).
  - normalization: reciprocal of the sums row, partition-broadcast via a
    tiny DRAM roundtrip, multiplied in during the ctx psum eviction.
  - output projection consumes ctx_normT [e, s] directly; the bv/bo
    contribution is a constant row (softmax rows sum to 1) added on host.

All matmul operands are bfloat16 (1 cycle/row on the PE at any moving
size, half the DMA bytes / SBUF of f32r). PSUM accumulation stays fp32;
softmax denominators come from the same bf16 e values as the numerators,
so normalization is consistent. End-to-end rel err ~5.7e-3 vs fp32
(gate: 2e-2). Output is written bf16 and upcast on host.

Scheduling notes (sim-guided, CoreSim cost model; body ~302us sim):
  - only the 12 per-head ones-columns of V are memset (a full-V memset
    occupies DVE ~10us at startup and stalls the first q-block eviction);
  - Wq/Wk ship host-swizzled as [NE, 128, D] so each per-pair weight
    load is one contiguous DMA (column-block loads of a [D, D] matrix
    pay a 2x sub-512B-row DMA penalty); pair 0's weights preload ahead
    of the xT stream, whose first q-block alternates both DMA queues;
    later q-blocks load as single strided DMAs (descriptor generation
    costs ~500ns per dma_start per queue);
  - woT loads after xT on the same queue (first needed only by phase3);
  - normalization: reciprocal straight off the PSUM sums row (bf16),
    DRAM broadcast roundtrip + Pool-engine multiply in steady state
    (GpSimd cannot touch PSUM, so eviction copies stay on DVE); the
    body-tail unit instead broadcasts via a rank-1 PE matmul
    (ones[1,DH]^T @ rc -> PSUM) to dodge the shared-DMA-engine backlog,
    evicts ctx on the post-exp-idle ScalarE, and multiplies on DVE;
  - output evictions alternate DMA queues (gpsimd/sync); the tail
    q-block also alternates its eviction copies across ScalarE/DVE.
"""

import numpy as np
from contextlib import ExitStack

import concourse.bass as bass
import concourse.bacc as bacc
import concourse.tile as tile
from concourse import mybir
from concourse import bass_utils

F32 = mybir.dt.float32
F32R = mybir.dt.float32r
BF16 = mybir.dt.bfloat16
CDT = BF16               # compute dtype for matmul operands (1 cyc/row)
NPCDT = mybir.dt.np(CDT)
AF = mybir.ActivationFunctionType
OP = mybir.AluOpType

P = 128
D = 768
H = 12
DH = 64
NE = D // P          # 6 e-chunks (head pairs)
ND = D // P          # 6 d-chunks
SCALE = 0.125
S_FULL = 1500
QB = 512
EH = 384             # half of D for the V projection moving dim


def _chunks(total, size):
    out = []
    o = 0
    while o < total:
        out.append((o, min(size, total - o)))
        o += size
    return out


def _qblocks(S):
    """512-wide q-blocks; the last one is narrower (phase 2 handles qw<512
    with split exp instructions). An overlapped uniform-512 last block was
    tried (-9us ACT from single exps) but created +14us of pipeline idle
    -- net loss, reverted."""
    return _chunks(S, QB)


def build_attention(tc, ctx, xT, wqT, wkT, wvT, woT, bqs, ident, out, S,
                    reps=1,
                    hw_loop=False):
    """Emit the single-core attention program.

    xT:  [D, S] f32r DRAM     (x^T for this batch element)
    wqT/wkT/wvT/woT: [D, D] f32r DRAM  (W.T of the torch-Linear weights)
    bqs: [P, NE] f32 DRAM     (0.125*bq laid out [partition, e-chunk])
    out: [S, D] f32 DRAM      (missing the constant bv@Wo.T+bo row)
    """
    nc = tc.nc
    SC = _chunks(S, P)            # k-chunks, e.g. 11x128 + 92
    QBS = _qblocks(S)
    NSC = len(SC)

    const = ctx.enter_context(tc.tile_pool(name="const", bufs=1))
    qkv = ctx.enter_context(tc.tile_pool(name="qkv", bufs=1))
    gen_ps = ctx.enter_context(tc.tile_pool(name="gen_ps", bufs=2, space="PSUM"))
    sc_ps = ctx.enter_context(tc.tile_pool(name="sc_ps", bufs=2, space="PSUM"))
    ctx_ps = ctx.enter_context(tc.tile_pool(name="ctx_ps", bufs=2, space="PSUM"))
    e_pool = ctx.enter_context(tc.tile_pool(name="epool", bufs=3))
    ctxn_pool = ctx.enter_context(tc.tile_pool(name="ctxn", bufs=3))
    craw_pool = ctx.enter_context(tc.tile_pool(name="craw", bufs=4))
    rb_pool = ctx.enter_context(tc.tile_pool(name="rbp", bufs=4))
    out_sb_pool = ctx.enter_context(tc.tile_pool(name="outsb", bufs=8))
    p3part_pool = ctx.enter_context(tc.tile_pool(name="p3part", bufs=1))
    qt_rd = ctx.enter_context(tc.tile_pool(name="qtrd", bufs=4))
    qt_st = ctx.enter_context(tc.tile_pool(name="qtst", bufs=2))
    dram = ctx.enter_context(tc.tile_pool(name="dram", bufs=8, space="DRAM"))
    dram1 = ctx.enter_context(tc.tile_pool(name="dram1", bufs=1, space="DRAM"))

    # Persistent operands
    kt_pool = ctx.enter_context(tc.tile_pool(name="ktp", bufs=2))
    V = qkv.tile([P, NSC, H * (DH + 1)], CDT)   # per-head 65th ones column
    qt_dram = dram1.tile([D, S], CDT)
    bq_sb = const.tile([P, NE], F32)
    # ScalarE's DMA queue is empty at startup; keeps the sync queue free
    # for the first pair's weight load (the first matmul's stationary
    # operand).
    nc.scalar.dma_start(out=bq_sb[:], in_=bqs)
    woT_sb = const.tile([P, NE, D], CDT)

    # Initialize only the per-head 65th columns to 1.0 (the softmax
    # denominator accumulators); the 64 data columns per head are fully
    # overwritten by the V-projection evictions. Memsetting all of V
    # would occupy DVE for ~10us at startup and stall the first q-block
    # eviction behind it.
    v4 = V[:, :, :].rearrange("p n (h w) -> p n h w", w=DH + 1)
    nc.vector.memset(v4[:, :, :, DH], 1.0)
    ident_sb = const.tile([P, P], CDT)
    nc.sync.dma_start(out=ident_sb[:], in_=ident)

    if hw_loop:
        # Hardware loop: NEFF size is independent of reps, so wall-clock
        # slope vs reps isolates pure device execution time.
        with tc.For_i(0, reps, 1):
            _emit_body(tc, nc, xT, wqT, wkT, wvT, woT, out, S, SC, QBS, NSC,
                       kt_pool, V, qt_dram, bq_sb, woT_sb, ident_sb,
                       gen_ps, sc_ps, ctx_ps, e_pool, ctxn_pool, craw_pool,
                       rb_pool, out_sb_pool, p3part_pool, qt_rd, qt_st, dram)
    else:
        for _rep in range(reps):
            _emit_body(tc, nc, xT, wqT, wkT, wvT, woT, out, S, SC, QBS, NSC,
                       kt_pool, V, qt_dram, bq_sb, woT_sb, ident_sb,
                       gen_ps, sc_ps, ctx_ps, e_pool, ctxn_pool, craw_pool,
                       rb_pool, out_sb_pool, p3part_pool, qt_rd, qt_st, dram)


def _emit_body(tc, nc, xT, wqT, wkT, wvT, woT, out, S, SC, QBS, NSC,
               kt_pool, V, qt_dram, bq_sb, woT_sb, ident_sb,
               gen_ps, sc_ps, ctx_ps, e_pool, ctxn_pool, craw_pool,
               rb_pool, out_sb_pool, p3part_pool, qt_rd, qt_st, dram):

    def phase2_begin(q0, qw, pr, kt_t, qts):
        return (q0, qw, pr, kt_t, qts[q0], [])

    def phase2_kc(st, kc):
        (q0, qw, pr, kt_t, qt_sb, es) = st
        (k0, kw) = SC[kc]
        sp = sc_ps.tile([P, 1024], F32, tag="sc", name="sp")
        # hi*512 offsets: a matmul output may not cross the 512-f32 PSUM
        # bank boundary, so the narrow last q-block cannot pack the two
        # heads contiguously and pays a split exp instead.
        for hi in range(2):
            nc.tensor.matmul(
                sp[:kw, hi * 512:hi * 512 + qw],
                kt_t[hi * DH:(hi + 1) * DH, k0:k0 + kw],
                qt_sb[hi * DH:(hi + 1) * DH, 0:qw],
                start=True, stop=True)
        e_sb = e_pool.tile([P, 1024], CDT, tag=f"e{kc}", name=f"e_sb{kc}")
        # NOTE: a single exp spanning the [qw, 512) gap between the heads
        # for narrow blocks is ILLEGAL -- the gap belongs to another tile
        # generation, so the ACT read races a concurrent score matmul's
        # PE write to the same bank region (fatal hazard class).
        if qw == 512:
            nc.scalar.activation(out=e_sb[:kw, :], in_=sp[:kw, :],
                                 func=AF.Exp)
        else:
            for hi in range(2):
                nc.scalar.activation(
                    out=e_sb[:kw, hi * 512:hi * 512 + qw],
                    in_=sp[:kw, hi * 512:hi * 512 + qw], func=AF.Exp)
        es.append(e_sb)

    def phase2_ctx(st, cn):
        """probs @ V with e as STATIONARY [k, q<=128] and V as MOVING
        [k, DH+1]: all 128 output partitions used per pass at 65 moving
        rows per (k-chunk, head) -- less than half the PE rows of the
        V-stationary form (512 rows). The softmax denominator lands as a
        per-PARTITION scalar (q is the partition dim), so normalization
        is a plain tensor_scalar -- no partition broadcast at all -- and
        a PE transpose restores the [e, s] layout phase3 consumes."""
        (q0, qw, pr, kt_t, qt_sb, es) = st
        for (qs0, qsw) in _chunks(qw, P):
            cnT = craw_pool.tile([P, 2 * DH], CDT, tag="cnT", name="cnT")
            for hi in range(2):
                h = 2 * pr + hi
                cps = ctx_ps.tile([P, DH + 1], F32, tag="ctx", name="cps")
                for kc in range(NSC):
                    (k0, kw) = SC[kc]
                    nc.tensor.matmul(
                        cps[:qsw, :],
                        es[kc][:kw, hi * 512 + qs0:hi * 512 + qs0 + qsw],
                        V[:kw, kc, h * (DH + 1):(h + 1) * (DH + 1)],
                        start=(kc == 0), stop=(kc == NSC - 1))
                rcol = craw_pool.tile([P, 1], F32, tag="rc", name="rcol")
                nc.vector.reciprocal(out=rcol[:qsw, :],
                                     in_=cps[:qsw, DH:DH + 1])
                with nc.allow_low_precision(reason="bf16 normalized ctx"):
                    nc.vector.tensor_scalar_mul(
                        cnT[:qsw, hi * DH:(hi + 1) * DH],
                        cps[:qsw, 0:DH], rcol[:qsw, :])
            # Both heads' normalized tiles share one [qsw, 128] SBUF tile,
            # so a SINGLE transpose+eviction restores the full [e, s]
            # slice -- half the transpose rows of per-head transposes.
            tp = gen_ps.tile([P, P], CDT, tag="mm", name="tp")
            nc.tensor.matmul(tp[:, 0:qsw], cnT[:qsw, 0:2 * DH],
                             ident_sb[:qsw, 0:qsw], is_transpose=True)
            nc.vector.tensor_copy(
                out=cn[:, pr, qs0:qs0 + qsw],
                in_=tp[:, 0:qsw])

    def phase2_scores(q0, qw, pr, kt_t, qts):
        st = phase2_begin(q0, qw, pr, kt_t, qts)
        for kc in range(NSC):
            phase2_kc(st, kc)
        return st

    def phase3_partials(q0, qw, cn, s_sel=None, s_off=0):
        # Pairs 0..NE-2 of the output projection for this q-block: their
        # ctx slices were written during those pairs' own iterations, so
        # these accumulations are ready LONG before the last pair -- emit
        # them early (priority-wise) so they soak up mid-stream PE gaps
        # instead of serializing in the tail.
        parts = {}
        schunks = _chunks(qw, P)
        if s_off:
            schunks[0] = (s_off, schunks[0][1] - s_off)
        for si, (s0, sw) in enumerate(schunks):
            if s_sel is not None and si not in s_sel:
                continue
            for (o0, ow) in ((0, 512), (512, 256)):
                op_t = gen_ps.tile([P, 512], F32, tag="mm", name="op_t")
                for ec in range(NE - 2):
                    nc.tensor.matmul(
                        op_t[:sw, :ow],
                        cn[:, ec, s0:s0 + sw],
                        woT_sb[:, ec, o0:o0 + ow],
                        start=(ec == 0), stop=(ec == NE - 3))
                part = p3part_pool.tile([P, 512], F32, tag=f"p3p{si}{o0}",
                                        name="p3part")
                nc.vector.tensor_copy(out=part[:sw, :ow],
                                      in_=op_t[:sw, :ow])
                parts[(si, o0)] = part
        return parts

    def phase3_final(q0, qw, cn, parts, s_sel=None, s_off=0):
        # Only the last pair's single accumulation + the add-eviction
        # remain on the critical tail.
        schunks = _chunks(qw, P)
        if s_off:
            schunks[0] = (s_off, schunks[0][1] - s_off)
        for si, (s0, sw) in enumerate(schunks):
            if s_sel is not None and si not in s_sel:
                continue
            for (o0, ow) in ((0, 512), (512, 256)):
                op5 = gen_ps.tile([P, 512], F32, tag="mm", name="op5")
                for ec in (NE - 2, NE - 1):
                    nc.tensor.matmul(
                        op5[:sw, :ow],
                        cn[:, ec, s0:s0 + sw],
                        woT_sb[:, ec, o0:o0 + ow],
                        start=(ec == NE - 2), stop=(ec == NE - 1))
                ot = out_sb_pool.tile([P, 512], CDT, tag="ot", name="ot")
                with nc.allow_low_precision(reason="bf16 output"):
                    nc.vector.tensor_tensor(
                        out=ot[:sw, :ow], in0=parts[(si, o0)][:sw, :ow],
                        in1=op5[:sw, :ow], op=OP.add)
                par = (si * 2 + (o0 > 0)) % 2
                deng = nc.gpsimd if par == 0 else nc.sync
                deng.dma_start(out=out[q0 + s0:q0 + s0 + sw, o0:o0 + ow],
                               in_=ot[:sw, :ow])

    def phase3(q0, qw, cn, s_sel=None, split_last=False, tail=False):
        schunks = _chunks(qw, P)
        for si, (s0, sw) in enumerate(schunks):
            if s_sel is not None and si not in s_sel:
                continue
            for (o0, ow) in ((0, 512), (512, 256)):
                ot = out_sb_pool.tile([P, 512], CDT, tag="ot", name="ot")
                par = (si * 2 + (o0 > 0)) % 2
                if split_last:
                    # Two-phase accumulation: pairs 0..NE-2 finish and evict
                    # to SBUF without waiting for the LAST pair's
                    # normalization chain; pair NE-1 joins via a single late
                    # matmul folded into the final (add) eviction. Keeps
                    # gen_ps buffers free while the chain is in flight.
                    op_t = gen_ps.tile([P, 512], F32, tag="mm", name="op_t")
                    for ec in range(NE - 1):
                        nc.tensor.matmul(
                            op_t[:sw, :ow],
                            cn[:, ec, s0:s0 + sw],
                            woT_sb[:, ec, o0:o0 + ow],
                            start=(ec == 0), stop=(ec == NE - 2))
                    part = p3part_pool.tile([P, 512], F32, tag="p3p",
                                            name="p3part")
                    nc.vector.tensor_copy(out=part[:sw, :ow],
                                          in_=op_t[:sw, :ow])
                    op5 = gen_ps.tile([P, 512], F32, tag="mm", name="op5")
                    nc.tensor.matmul(
                        op5[:sw, :ow],
                        cn[:, NE - 1, s0:s0 + sw],
                        woT_sb[:, NE - 1, o0:o0 + ow],
                        start=True, stop=True)
                    with nc.allow_low_precision(reason="bf16 output"):
                        nc.vector.tensor_tensor(
                            out=ot[:sw, :ow], in0=part[:sw, :ow],
                            in1=op5[:sw, :ow], op=OP.add)
                else:
                    op_t = gen_ps.tile([P, 512], F32, tag="mm", name="op_t")
                    for ec in range(NE):
                        nc.tensor.matmul(
                            op_t[:sw, :ow],
                            cn[:, ec, s0:s0 + sw],
                            woT_sb[:, ec, o0:o0 + ow],
                            start=(ec == 0), stop=(ec == NE - 1))
                    with nc.allow_low_precision(reason="bf16 output"):
                        nc.vector.tensor_copy(out=ot[:sw, :ow],
                                              in_=op_t[:sw, :ow])
                deng = nc.gpsimd if par == 0 else nc.sync
                deng.dma_start(out=out[q0 + s0:q0 + s0 + sw, o0:o0 + ow],
                               in_=ot[:sw, :ow])

    with tc.tile_pool(name="xw", bufs=1) as xw, \
         tc.tile_pool(name="wecp", bufs=2) as wecp:
        xT_sb = xw.tile([P, ND, S], CDT)
        # Pair 0's q-weights go first on the sync queue so the very first
        # projection matmul has its stationary operand by ~1us; the first
        # q-block's xT chunks alternate across both DMA queues so they
        # arrive at ~2x the single-queue rate.
        pre0 = {"q": wecp.tile([P, ND, P], CDT, tag="wec", name="wec_q0")}
        nc.sync.dma_start(out=pre0["q"][:, :, :], in_=wqT[0])
        for qi, (q0, qw) in enumerate(QBS):
            if qi == 0:
                # fine-grained + dual-queue so the first projection's
                # operands land as early as possible
                for dc in range(ND):
                    eng = nc.sync if dc % 2 == 1 else nc.gpsimd
                    eng.dma_start(
                        out=xT_sb[:, dc, q0:q0 + qw],
                        in_=xT[dc * P:(dc + 1) * P, q0:q0 + qw])
                pre0["k"] = wecp.tile([P, ND, P], CDT, tag="wec",
                                      name="wec_k0")
                nc.sync.dma_start(out=pre0["k"][:, :, :], in_=wkT[0])
            else:
                # one strided DMA per q-block: descriptor generation is
                # ~500ns per dma_start, so 6 small loads would throttle
                # the Pool queue for the rest of the startup
                nc.gpsimd.dma_start(
                    out=xT_sb[:, :, q0:q0 + qw],
                    in_=xT.rearrange("(c p) s -> p c s", p=P)[
                        :, :, q0:q0 + qw])
        # woT loads behind xT on the same queue: first needed by phase3,
        # which only runs near the end of the body.
        nc.gpsimd.dma_start(out=woT_sb[:, :, :],
                            in_=woT.rearrange("(c p) e -> p c e", p=P))

        def emit_kq(ec, pre=None):
            kt_t = kt_pool.tile([P, S], CDT, tag="kt", name=f"kt{ec}")
            qts = {}
            wecs = {}
            for kind, wdram in (("q", wqT), ("k", wkT)):
                if pre is not None and kind in pre:
                    wecs[kind] = pre[kind]
                else:
                    wecs[kind] = wecp.tile([P, ND, P], CDT, tag="wec",
                                           name=f"wec_{kind}{ec}")
                    nc.sync.dma_start(out=wecs[kind][:, :, :], in_=wdram[ec])
            # q-block-major: the first scores need q AND k of q-block 0,
            # so emit q(qb0), k(qb0) before the later q-blocks instead of
            # all of q before all of k. K uses NON-overlapping blocks
            # (an overlapped last block would double-write kt columns and
            # serialize every unit's scores on that eviction); only Q
            # needs the uniform-512 overlap for the single-exp win.
            kblocks = _chunks(S, QB)
            for bi in range(len(QBS)):
                for kind in ("q", "k"):
                    (q0, qw) = QBS[bi] if kind == "q" else kblocks[bi]
                    wec = wecs[kind]
                    ps = gen_ps.tile([P, 512], F32, tag="mm", name="kq_ps")
                    for dc in range(ND):
                        nc.tensor.matmul(
                            ps[:, :qw],
                            wec[:, dc, :],
                            xT_sb[:, dc, q0:q0 + qw],
                            start=(dc == 0), stop=(dc == ND - 1))
                    if kind == "q":
                        # Each pair's Q is consumed only by this pair's own
                        # phase2, which follows immediately -- keep it
                        # SBUF-resident instead of the legacy DRAM
                        # roundtrip (36 DMAs + latency on the first score).
                        qs = qt_st.tile([P, 512], CDT, tag=f"qs{q0}",
                                        name=f"qs_{ec}_{q0}")
                        nc.vector.tensor_scalar(
                            out=qs[:, 0:qw], in0=ps[:, :qw],
                            scalar1=SCALE, scalar2=bq_sb[:, ec:ec + 1],
                            op0=OP.mult, op1=OP.add)
                        qts[q0] = qs
                    else:
                        nc.vector.tensor_copy(out=kt_t[:, q0:q0 + qw],
                                              in_=ps[:, :qw])
            return kt_t, qts

        def emit_v_chunk(w_sb, sc, s0, sw):
            for eh in range(D // EH):
                ps = gen_ps.tile([P, 512], F32, tag="mm", name="v_ps")
                for dc in range(ND):
                    nc.tensor.matmul(
                        ps[:sw, :EH],
                        xT_sb[:, dc, s0:s0 + sw],
                        w_sb[:, dc, eh * EH:(eh + 1) * EH],
                        start=(dc == 0), stop=(dc == ND - 1))
                vh = V[:sw, sc, :].rearrange("p (h w) -> p h w", w=DH + 1)
                nc.vector.tensor_copy(
                    out=vh[:, eh * (EH // DH):(eh + 1) * (EH // DH), 0:DH],
                    in_=ps[:sw, :EH].rearrange("p (h w) -> p h w", w=DH))

        # pr-major emission (emission order IS program order under Tile):
        # each head-pair's K/Q projection is followed by that pair's
        # attention over ALL q-blocks, so the 6 projection units spread
        # across 18 ACT-bound attention units and ScalarE stays the pacer.
        # The V pass interleaves chunk-by-chunk with the very first pair so
        # exp work starts within ~20us of kernel start. Each q-block's
        # output projection is emitted right after its last pair.
        cns = [ctxn_pool.tile([P, NE, 512], CDT, tag="cn", name=f"cn{_q}")
               for _q in range(len(QBS))]
        # Software pipeline: each unit's ctx phase (PE-cheap, DVE-chained)
        # is emitted AFTER the NEXT unit's scores, so the in-order PE
        # stream always hands ACT its 12 exps before grinding the
        # previous unit's ctx/evictions -- otherwise ACT starves ~5us at
        # every q-block boundary.
        pending = []

        def flush_pending():
            while pending:
                st_, cn_ = pending.pop(0)
                phase2_ctx(st_, cn_)

        for pr in range(NE):
            kt_t, qts = emit_kq(pr, pre=pre0 if pr == 0 else None)
            for qi, (q0, qw) in enumerate(QBS):
                if pr == 0 and qi <= 1:
                    # Spread the V projection (23us of PE) over the first
                    # TWO units' k-chunk interleave: packed into one unit
                    # it paces that unit's exps at ~2.3us instead of
                    # ~1.05us and ACT idles ~15us. The deferred-ctx
                    # pipeline flushes ctx(unit0) only after unit1's
                    # scores, so V still completes in time.
                    st = phase2_begin(q0, qw, 0, kt_t, qts)
                    if qi == 0:
                        w_sb = xw.tile([P, ND, D], CDT, tag="w",
                                       name="w_sb")
                        for dc in range(ND):
                            nc.gpsimd.dma_start(
                                out=w_sb[:, dc, :],
                                in_=wvT[dc * P:(dc + 1) * P, :])
                    half_v = (NSC + 1) // 2
                    for kc in range(NSC):
                        if kc % 2 == 0:
                            sc = qi * half_v + kc // 2
                            if sc < NSC:
                                (s0, sw) = SC[sc]
                                emit_v_chunk(w_sb, sc, s0, sw)
                        phase2_kc(st, kc)
                else:
                    st = phase2_scores(q0, qw, pr, kt_t, qts)
                pending.append((st, cns[qi]))
                ql0, qlw = QBS[-1]
                ov = (QBS[-2][0] + QBS[-2][1]) - ql0 if len(QBS) > 1 else 0
                if pr == NE - 2 and qi == 0:
                    # pairs 0..NE-3 have finished the last q-block's ctx by
                    # now (the deferred pipeline is one unit behind): emit
                    # the partial output accumulations here so they fill
                    # PE gaps across the last TWO pairs' stretches.
                    flush_pending()
                    q2parts = phase3_partials(ql0, qlw, cns[-1], s_off=ov)
                if pr == NE - 1:
                    flush_pending()
                    # Interleave: finish the PREVIOUS q-block's second half
                    # here so its PE work covers this block's normalization
                    # chain; the current block keeps its first half only.
                    nsc_q = len(_chunks(qw, P))
                    half = nsc_q // 2
                    if qi > 0:
                        pq0, pqw = QBS[qi - 1]
                        pn = len(_chunks(pqw, P))
                        phase3(pq0, pqw, cns[qi - 1],
                               s_sel=set(range(pn // 2, pn)))
                    if qi < len(QBS) - 1:
                        phase3(q0, qw, cns[qi], s_sel=set(range(half)))
                    else:
                        phase3_final(q0, qw, cns[qi], q2parts, s_off=ov)
                elif len(pending) > 1:
                    phase2_ctx(*pending.pop(0))


def build_nc(S=S_FULL, reps=1, hw_loop=False):
    nc = bacc.Bacc("TRN2", target_bir_lowering=False, debug=False,
                   enable_asserts=False, num_devices=1)
    xT = nc.dram_tensor("xT", [D, S], CDT, kind="ExternalInput").ap()
    wqT = nc.dram_tensor("wqT", [NE, P, D], CDT, kind="ExternalInput").ap()
    wkT = nc.dram_tensor("wkT", [NE, P, D], CDT, kind="ExternalInput").ap()
    wvT = nc.dram_tensor("wvT", [D, D], CDT, kind="ExternalInput").ap()
    woT = nc.dram_tensor("woT", [D, D], CDT, kind="ExternalInput").ap()
    bqs = nc.dram_tensor("bqs", [P, NE], F32, kind="ExternalInput").ap()
    ident = nc.dram_tensor("ident", [P, P], CDT, kind="ExternalInput").ap()
    out = nc.dram_tensor("out", [S, D], CDT, kind="ExternalOutput").ap()
    with tile.TileContext(nc) as tc:
        with ExitStack() as ctx:
            build_attention(tc, ctx, xT, wqT, wkT, wvT, woT, bqs, ident,
                            out, S, reps, hw_loop)
    nc.compile()
    return nc


_NC_CACHE = {}


def _get_nc(S=S_FULL, reps=1, hw_loop=False):
    key = (S, reps, hw_loop)
    if key not in _NC_CACHE:
        _NC_CACHE[key] = build_nc(S, reps, hw_loop)
    return _NC_CACHE[key]


def prep_inputs(x, Wq, bq, Wk, Wv, bv, Wo, bo):
    x = np.asarray(x, dtype=np.float32)
    Wq = np.asarray(Wq, dtype=np.float32)
    Wk = np.asarray(Wk, dtype=np.float32)
    Wv = np.asarray(Wv, dtype=np.float32)
    Wo = np.asarray(Wo, dtype=np.float32)
    bq = np.asarray(bq, dtype=np.float32)
    bv = np.asarray(bv, dtype=np.float32)
    bo = np.asarray(bo, dtype=np.float32)
    xT = np.ascontiguousarray(x.transpose(0, 2, 1)).astype(NPCDT)
    def _sw(wt):
        # [D, D] -> [NE, P, ND*P]: block-column ec, gathered over dc rows
        return np.ascontiguousarray(
            wt.reshape(ND, P, NE, P).transpose(2, 1, 0, 3).reshape(NE, P, D))
    base = {
        "wqT": _sw(Wq.T).astype(NPCDT),
        "wkT": _sw(Wk.T).astype(NPCDT),
        "wvT": np.ascontiguousarray(Wv.T).astype(NPCDT),
        "woT": np.ascontiguousarray(Wo.T).astype(NPCDT),
        "bqs": np.ascontiguousarray((SCALE * bq).reshape(NE, P).T),
        "ident": np.eye(P, dtype=np.float32).astype(NPCDT),
    }
    const_row = (bv @ Wo.T + bo).astype(np.float32)
    in_maps = [dict(base, xT=np.ascontiguousarray(xT[b])) for b in range(x.shape[0])]
    return in_maps, const_row


def kernel(x, Wq, bq, Wk, Wv, bv, Wo, bo):
    in_maps, const_row = prep_inputs(x, Wq, bq, Wk, Wv, bv, Wo, bo)
    nc = _get_nc(x.shape[1])
    res = bass_utils.run_bass_kernel_spmd(
        nc, in_maps, core_ids=list(range(len(in_maps))))
    out = np.stack([np.asarray(r["out"], dtype=np.float32)
                    for r in res.results])
    return (out + const_row[None, None, :]).astype(np.float32)



# revision 70
# speedup vs baseline: 1.1964x; 1.0321x over previous
"""Trainium2 Bass kernel for batched multi-head self-attention.

Problem: x [8, 1500, 768], 12 heads x 64 dims, torch-Linear style projections.
Strategy: data-parallel over batch (1 element per NeuronCore, 8 cores).

Per-core design (host pre-transposes everything; device does no transposes):
  - xT [768, 1500]: projections contract over d on the partition axis.
  - Q^T, K^T in [e, s] layout (pairs of heads per 128-partition chunk).
    K^T and V stay SBUF-resident; Q^T roundtrips through a DRAM scratch
    (each [head-pair, q-block] slice is consumed exactly once).
  - q-blocks are uniformly 512 wide; the last block overlaps the previous
    one (start S-512) so no padding or edge cases exist for S >= 512.
  - scores computed TRANSPOSED: scoresT[k, q] = K_h^T.T @ Q_h^T, two heads
    per PE pass via row tile_position packing (contraction is dh=64 only).
  - exp on ScalarE straight out of PSUM ([128,1024] two-bank spans), no max
    subtraction (scores ~ N(0,1): fp32-safe).
  - softmax denominators ride as a 65th all-ones column of V inside the ctx
    matmul (ctxT psum = 64 ctx rows + 1 sums row).
  - normalization: reciprocal of the sums row, partition-broadcast via a
    tiny DRAM roundtrip, multiplied in during the ctx psum eviction.
  - output projection consumes ctx_normT [e, s] directly; the bv/bo
    contribution is a constant row (softmax rows sum to 1) added on host.

All matmul operands are bfloat16 (1 cycle/row on the PE at any moving
size, half the DMA bytes / SBUF of f32r). PSUM accumulation stays fp32;
softmax denominators come from the same bf16 e values as the numerators,
so normalization is consistent. End-to-end rel err ~5.7e-3 vs fp32
(gate: 2e-2). Output is written bf16 and upcast on host.

Scheduling notes (sim-guided, CoreSim cost model; body ~302us sim):
  - only the 12 per-head ones-columns of V are memset (a full-V memset
    occupies DVE ~10us at startup and stalls the first q-block eviction);
  - Wq/Wk ship host-swizzled as [NE, 128, D] so each per-pair weight
    load is one contiguous DMA (column-block loads of a [D, D] matrix
    pay a 2x sub-512B-row DMA penalty); pair 0's weights preload ahead
    of the xT stream, whose first q-block alternates both DMA queues;
    later q-blocks load as single strided DMAs (descriptor generation
    costs ~500ns per dma_start per queue);
  - woT loads after xT on the same queue (first needed only by phase3);
  - normalization: reciprocal straight off the PSUM sums row (bf16),
    DRAM broadcast roundtrip + Pool-engine multiply in steady state
    (GpSimd cannot touch PSUM, so eviction copies stay on DVE); the
    body-tail unit instead broadcasts via a rank-1 PE matmul
    (ones[1,DH]^T @ rc -> PSUM) to dodge the shared-DMA-engine backlog,
    evicts ctx on the post-exp-idle ScalarE, and multiplies on DVE;
  - output evictions alternate DMA queues (gpsimd/sync); the tail
    q-block also alternates its eviction copies across ScalarE/DVE.
"""

import numpy as np
from contextlib import ExitStack

import concourse.bass as bass
import concourse.bacc as bacc
import concourse.tile as tile
from concourse import mybir
from concourse import bass_utils

F32 = mybir.dt.float32
F32R = mybir.dt.float32r
BF16 = mybir.dt.bfloat16
CDT = BF16               # compute dtype for matmul operands (1 cyc/row)
NPCDT = mybir.dt.np(CDT)
AF = mybir.ActivationFunctionType
OP = mybir.AluOpType

P = 128
D = 768
H = 12
DH = 64
NE = D // P          # 6 e-chunks (head pairs)
ND = D // P          # 6 d-chunks
SCALE = 0.125
S_FULL = 1500
QB = 512
EH = 384             # half of D for the V projection moving dim


def _chunks(total, size):
    out = []
    o = 0
    while o < total:
        out.append((o, min(size, total - o)))
        o += size
    return out


def _qblocks(S):
    """512-wide q-blocks; the last one is narrower (phase 2 handles qw<512
    with split exp instructions). An overlapped uniform-512 last block was
    tried (-9us ACT from single exps) but created +14us of pipeline idle
    -- net loss, reverted."""
    return _chunks(S, QB)


def build_attention(tc, ctx, xT, wqT, wkT, wvT, woT, bqs, ident, out, S,
                    reps=1,
                    hw_loop=False):
    """Emit the single-core attention program.

    xT:  [D, S] f32r DRAM     (x^T for this batch element)
    wqT/wkT/wvT/woT: [D, D] f32r DRAM  (W.T of the torch-Linear weights)
    bqs: [P, NE] f32 DRAM     (0.125*bq laid out [partition, e-chunk])
    out: [S, D] f32 DRAM      (missing the constant bv@Wo.T+bo row)
    """
    nc = tc.nc
    SC = _chunks(S, P)            # k-chunks, e.g. 11x128 + 92
    QBS = _qblocks(S)
    NSC = len(SC)

    const = ctx.enter_context(tc.tile_pool(name="const", bufs=1))
    qkv = ctx.enter_context(tc.tile_pool(name="qkv", bufs=1))
    gen_ps = ctx.enter_context(tc.tile_pool(name="gen_ps", bufs=2, space="PSUM"))
    sc_ps = ctx.enter_context(tc.tile_pool(name="sc_ps", bufs=2, space="PSUM"))
    ctx_ps = ctx.enter_context(tc.tile_pool(name="ctx_ps", bufs=2, space="PSUM"))
    e_pool = ctx.enter_context(tc.tile_pool(name="epool", bufs=3))
    ctxn_pool = ctx.enter_context(tc.tile_pool(name="ctxn", bufs=3))
    craw_pool = ctx.enter_context(tc.tile_pool(name="craw", bufs=4))
    rb_pool = ctx.enter_context(tc.tile_pool(name="rbp", bufs=4))
    out_sb_pool = ctx.enter_context(tc.tile_pool(name="outsb", bufs=8))
    p3part_pool = ctx.enter_context(tc.tile_pool(name="p3part", bufs=1))
    qt_rd = ctx.enter_context(tc.tile_pool(name="qtrd", bufs=4))
    qt_st = ctx.enter_context(tc.tile_pool(name="qtst", bufs=2))
    dram = ctx.enter_context(tc.tile_pool(name="dram", bufs=8, space="DRAM"))
    dram1 = ctx.enter_context(tc.tile_pool(name="dram1", bufs=1, space="DRAM"))

    # Persistent operands
    kt_pool = ctx.enter_context(tc.tile_pool(name="ktp", bufs=2))
    V = qkv.tile([P, NSC, H * (DH + 1)], CDT)   # per-head 65th ones column
    qt_dram = dram1.tile([D, S], CDT)
    bq_sb = const.tile([P, NE], F32)
    # ScalarE's DMA queue is empty at startup; keeps the sync queue free
    # for the first pair's weight load (the first matmul's stationary
    # operand).
    nc.scalar.dma_start(out=bq_sb[:], in_=bqs)
    woT_sb = const.tile([P, NE, D], CDT)

    # Initialize only the per-head 65th columns to 1.0 (the softmax
    # denominator accumulators); the 64 data columns per head are fully
    # overwritten by the V-projection evictions. Memsetting all of V
    # would occupy DVE for ~10us at startup and stall the first q-block
    # eviction behind it.
    v4 = V[:, :, :].rearrange("p n (h w) -> p n h w", w=DH + 1)
    nc.vector.memset(v4[:, :, :, DH], 1.0)
    ident_sb = const.tile([P, P], CDT)
    nc.sync.dma_start(out=ident_sb[:], in_=ident)

    if hw_loop:
        # Hardware loop: NEFF size is independent of reps, so wall-clock
        # slope vs reps isolates pure device execution time.
        with tc.For_i(0, reps, 1):
            _emit_body(tc, nc, xT, wqT, wkT, wvT, woT, out, S, SC, QBS, NSC,
                       kt_pool, V, qt_dram, bq_sb, woT_sb, ident_sb,
                       gen_ps, sc_ps, ctx_ps, e_pool, ctxn_pool, craw_pool,
                       rb_pool, out_sb_pool, p3part_pool, qt_rd, qt_st, dram)
    else:
        for _rep in range(reps):
            _emit_body(tc, nc, xT, wqT, wkT, wvT, woT, out, S, SC, QBS, NSC,
                       kt_pool, V, qt_dram, bq_sb, woT_sb, ident_sb,
                       gen_ps, sc_ps, ctx_ps, e_pool, ctxn_pool, craw_pool,
                       rb_pool, out_sb_pool, p3part_pool, qt_rd, qt_st, dram)


def _emit_body(tc, nc, xT, wqT, wkT, wvT, woT, out, S, SC, QBS, NSC,
               kt_pool, V, qt_dram, bq_sb, woT_sb, ident_sb,
               gen_ps, sc_ps, ctx_ps, e_pool, ctxn_pool, craw_pool,
               rb_pool, out_sb_pool, p3part_pool, qt_rd, qt_st, dram):

    def phase2_begin(q0, qw, pr, kt_t, qts):
        return (q0, qw, pr, kt_t, qts[q0], [])

    def phase2_kc(st, kc):
        (q0, qw, pr, kt_t, qt_sb, es) = st
        (k0, kw) = SC[kc]
        sp = sc_ps.tile([P, 1024], F32, tag="sc", name="sp")
        # hi*512 offsets: a matmul output may not cross the 512-f32 PSUM
        # bank boundary, so the narrow last q-block cannot pack the two
        # heads contiguously and pays a split exp instead.
        for hi in range(2):
            nc.tensor.matmul(
                sp[:kw, hi * 512:hi * 512 + qw],
                kt_t[hi * DH:(hi + 1) * DH, k0:k0 + kw],
                qt_sb[hi * DH:(hi + 1) * DH, 0:qw],
                start=True, stop=True)
        e_sb = e_pool.tile([P, 1024], CDT, tag=f"e{kc}", name=f"e_sb{kc}")
        # NOTE: a single exp spanning the [qw, 512) gap between the heads
        # for narrow blocks is ILLEGAL -- the gap belongs to another tile
        # generation, so the ACT read races a concurrent score matmul's
        # PE write to the same bank region (fatal hazard class).
        if qw == 512:
            nc.scalar.activation(out=e_sb[:kw, :], in_=sp[:kw, :],
                                 func=AF.Exp)
        else:
            for hi in range(2):
                nc.scalar.activation(
                    out=e_sb[:kw, hi * 512:hi * 512 + qw],
                    in_=sp[:kw, hi * 512:hi * 512 + qw], func=AF.Exp)
        es.append(e_sb)

    def phase2_ctx(st, cn):
        """probs @ V with e as STATIONARY [k, q<=128] and V as MOVING
        [k, DH+1]: all 128 output partitions used per pass at 65 moving
        rows per (k-chunk, head) -- less than half the PE rows of the
        V-stationary form (512 rows). The softmax denominator lands as a
        per-PARTITION scalar (q is the partition dim), so normalization
        is a plain tensor_scalar -- no partition broadcast at all -- and
        a PE transpose restores the [e, s] layout phase3 consumes."""
        (q0, qw, pr, kt_t, qt_sb, es) = st
        for (qs0, qsw) in _chunks(qw, P):
            cnT = craw_pool.tile([P, 2 * DH], CDT, tag="cnT", name="cnT")
            for hi in range(2):
                h = 2 * pr + hi
                cps = ctx_ps.tile([P, DH + 1], F32, tag="ctx", name="cps")
                for kc in range(NSC):
                    (k0, kw) = SC[kc]
                    nc.tensor.matmul(
                        cps[:qsw, :],
                        es[kc][:kw, hi * 512 + qs0:hi * 512 + qs0 + qsw],
                        V[:kw, kc, h * (DH + 1):(h + 1) * (DH + 1)],
                        start=(kc == 0), stop=(kc == NSC - 1))
                rcol = craw_pool.tile([P, 1], F32, tag="rc", name="rcol")
                nc.vector.reciprocal(out=rcol[:qsw, :],
                                     in_=cps[:qsw, DH:DH + 1])
                with nc.allow_low_precision(reason="bf16 normalized ctx"):
                    nc.vector.tensor_scalar_mul(
                        cnT[:qsw, hi * DH:(hi + 1) * DH],
                        cps[:qsw, 0:DH], rcol[:qsw, :])
            # Both heads' normalized tiles share one [qsw, 128] SBUF tile,
            # so a SINGLE transpose+eviction restores the full [e, s]
            # slice -- half the transpose rows of per-head transposes.
            tp = gen_ps.tile([P, P], CDT, tag="mm", name="tp")
            nc.tensor.matmul(tp[:, 0:qsw], cnT[:qsw, 0:2 * DH],
                             ident_sb[:qsw, 0:qsw], is_transpose=True)
            nc.vector.tensor_copy(
                out=cn[:, pr, qs0:qs0 + qsw],
                in_=tp[:, 0:qsw])

    def phase2_scores(q0, qw, pr, kt_t, qts):
        st = phase2_begin(q0, qw, pr, kt_t, qts)
        for kc in range(NSC):
            phase2_kc(st, kc)
        return st

    def phase3_partials(q0, qw, cn, s_sel=None, s_off=0):
        # Pairs 0..NE-2 of the output projection for this q-block: their
        # ctx slices were written during those pairs' own iterations, so
        # these accumulations are ready LONG before the last pair -- emit
        # them early (priority-wise) so they soak up mid-stream PE gaps
        # instead of serializing in the tail.
        parts = {}
        schunks = _chunks(qw, P)
        if s_off:
            schunks[0] = (s_off, schunks[0][1] - s_off)
        for si, (s0, sw) in enumerate(schunks):
            if s_sel is not None and si not in s_sel:
                continue
            for (o0, ow) in ((0, 512), (512, 256)):
                op_t = gen_ps.tile([P, 512], F32, tag="mm", name="op_t")
                for ec in range(NE - 2):
                    nc.tensor.matmul(
                        op_t[:sw, :ow],
                        cn[:, ec, s0:s0 + sw],
                        woT_sb[:, ec, o0:o0 + ow],
                        start=(ec == 0), stop=(ec == NE - 3))
                part = p3part_pool.tile([P, 512], F32, tag=f"p3p{si}{o0}",
                                        name="p3part")
                nc.vector.tensor_copy(out=part[:sw, :ow],
                                      in_=op_t[:sw, :ow])
                parts[(si, o0)] = part
        return parts

    def phase3_final(q0, qw, cn, parts, s_sel=None, s_off=0):
        # Only the last pair's single accumulation + the add-eviction
        # remain on the critical tail.
        schunks = _chunks(qw, P)
        if s_off:
            schunks[0] = (s_off, schunks[0][1] - s_off)
        for si, (s0, sw) in enumerate(schunks):
            if s_sel is not None and si not in s_sel:
                continue
            for (o0, ow) in ((0, 512), (512, 256)):
                op5 = gen_ps.tile([P, 512], F32, tag="mm", name="op5")
                for ec in (NE - 2, NE - 1):
                    nc.tensor.matmul(
                        op5[:sw, :ow],
                        cn[:, ec, s0:s0 + sw],
                        woT_sb[:, ec, o0:o0 + ow],
                        start=(ec == NE - 2), stop=(ec == NE - 1))
                ot = out_sb_pool.tile([P, 512], CDT, tag="ot", name="ot")
                with nc.allow_low_precision(reason="bf16 output"):
                    nc.vector.tensor_tensor(
                        out=ot[:sw, :ow], in0=parts[(si, o0)][:sw, :ow],
                        in1=op5[:sw, :ow], op=OP.add)
                par = (si * 2 + (o0 > 0)) % 2
                deng = nc.gpsimd if par == 0 else nc.sync
                deng.dma_start(out=out[q0 + s0:q0 + s0 + sw, o0:o0 + ow],
                               in_=ot[:sw, :ow])

    def phase3(q0, qw, cn, s_sel=None, split_last=False, tail=False):
        schunks = _chunks(qw, P)
        for si, (s0, sw) in enumerate(schunks):
            if s_sel is not None and si not in s_sel:
                continue
            for (o0, ow) in ((0, 512), (512, 256)):
                ot = out_sb_pool.tile([P, 512], CDT, tag="ot", name="ot")
                par = (si * 2 + (o0 > 0)) % 2
                if split_last:
                    # Two-phase accumulation: pairs 0..NE-2 finish and evict
                    # to SBUF without waiting for the LAST pair's
                    # normalization chain; pair NE-1 joins via a single late
                    # matmul folded into the final (add) eviction. Keeps
                    # gen_ps buffers free while the chain is in flight.
                    op_t = gen_ps.tile([P, 512], F32, tag="mm", name="op_t")
                    for ec in range(NE - 1):
                        nc.tensor.matmul(
                            op_t[:sw, :ow],
                            cn[:, ec, s0:s0 + sw],
                            woT_sb[:, ec, o0:o0 + ow],
                            start=(ec == 0), stop=(ec == NE - 2))
                    part = p3part_pool.tile([P, 512], F32, tag="p3p",
                                            name="p3part")
                    nc.vector.tensor_copy(out=part[:sw, :ow],
                                          in_=op_t[:sw, :ow])
                    op5 = gen_ps.tile([P, 512], F32, tag="mm", name="op5")
                    nc.tensor.matmul(
                        op5[:sw, :ow],
                        cn[:, NE - 1, s0:s0 + sw],
                        woT_sb[:, NE - 1, o0:o0 + ow],
                        start=True, stop=True)
                    with nc.allow_low_precision(reason="bf16 output"):
                        nc.vector.tensor_tensor(
                            out=ot[:sw, :ow], in0=part[:sw, :ow],
                            in1=op5[:sw, :ow], op=OP.add)
                else:
                    op_t = gen_ps.tile([P, 512], F32, tag="mm", name="op_t")
                    for ec in range(NE):
                        nc.tensor.matmul(
                            op_t[:sw, :ow],
                            cn[:, ec, s0:s0 + sw],
                            woT_sb[:, ec, o0:o0 + ow],
                            start=(ec == 0), stop=(ec == NE - 1))
                    with nc.allow_low_precision(reason="bf16 output"):
                        nc.vector.tensor_copy(out=ot[:sw, :ow],
                                              in_=op_t[:sw, :ow])
                deng = nc.gpsimd if par == 0 else nc.sync
                deng.dma_start(out=out[q0 + s0:q0 + s0 + sw, o0:o0 + ow],
                               in_=ot[:sw, :ow])

    with tc.tile_pool(name="xw", bufs=1) as xw, \
         tc.tile_pool(name="wecp", bufs=2) as wecp:
        xT_sb = xw.tile([P, ND, S], CDT)
        # Pair 0's q-weights go first on the sync queue so the very first
        # projection matmul has its stationary operand by ~1us; the first
        # q-block's xT chunks alternate across both DMA queues so they
        # arrive at ~2x the single-queue rate.
        pre0 = {"q": wecp.tile([P, ND, P], CDT, tag="wec", name="wec_q0")}
        nc.sync.dma_start(out=pre0["q"][:, :, :], in_=wqT[0])
        for qi, (q0, qw) in enumerate(QBS):
            if qi == 0:
                # fine-grained + dual-queue so the first projection's
                # operands land as early as possible
                for dc in range(ND):
                    eng = nc.sync if dc % 2 == 1 else nc.gpsimd
                    eng.dma_start(
                        out=xT_sb[:, dc, q0:q0 + qw],
                        in_=xT[dc * P:(dc + 1) * P, q0:q0 + qw])
                pre0["k"] = wecp.tile([P, ND, P], CDT, tag="wec",
                                      name="wec_k0")
                nc.sync.dma_start(out=pre0["k"][:, :, :], in_=wkT[0])
            else:
                # one strided DMA per q-block: descriptor generation is
                # ~500ns per dma_start, so 6 small loads would throttle
                # the Pool queue for the rest of the startup
                nc.gpsimd.dma_start(
                    out=xT_sb[:, :, q0:q0 + qw],
                    in_=xT.rearrange("(c p) s -> p c s", p=P)[
                        :, :, q0:q0 + qw])
        # woT loads behind xT on the same queue: first needed by phase3,
        # which only runs near the end of the body.
        nc.gpsimd.dma_start(out=woT_sb[:, :, :],
                            in_=woT.rearrange("(c p) e -> p c e", p=P))

        def emit_kq(ec, pre=None):
            kt_t = kt_pool.tile([P, S], CDT, tag="kt", name=f"kt{ec}")
            qts = {}
            wecs = {}
            for kind, wdram in (("q", wqT), ("k", wkT)):
                if pre is not None and kind in pre:
                    wecs[kind] = pre[kind]
                else:
                    wecs[kind] = wecp.tile([P, ND, P], CDT, tag="wec",
                                           name=f"wec_{kind}{ec}")
                    nc.sync.dma_start(out=wecs[kind][:, :, :], in_=wdram[ec])
            # q-block-major: the first scores need q AND k of q-block 0,
            # so emit q(qb0), k(qb0) before the later q-blocks instead of
            # all of q before all of k. K uses NON-overlapping blocks
            # (an overlapped last block would double-write kt columns and
            # serialize every unit's scores on that eviction); only Q
            # needs the uniform-512 overlap for the single-exp win.
            kblocks = _chunks(S, QB)
            for bi in range(len(QBS)):
                for kind in ("q", "k"):
                    (q0, qw) = QBS[bi] if kind == "q" else kblocks[bi]
                    wec = wecs[kind]
                    ps = gen_ps.tile([P, 512], F32, tag="mm", name="kq_ps")
                    for dc in range(ND):
                        nc.tensor.matmul(
                            ps[:, :qw],
                            wec[:, dc, :],
                            xT_sb[:, dc, q0:q0 + qw],
                            start=(dc == 0), stop=(dc == ND - 1))
                    if kind == "q":
                        # Each pair's Q is consumed only by this pair's own
                        # phase2, which follows immediately -- keep it
                        # SBUF-resident instead of the legacy DRAM
                        # roundtrip (36 DMAs + latency on the first score).
                        qs = qt_st.tile([P, 512], CDT, tag=f"qs{q0}",
                                        name=f"qs_{ec}_{q0}")
                        nc.vector.tensor_scalar(
                            out=qs[:, 0:qw], in0=ps[:, :qw],
                            scalar1=SCALE, scalar2=bq_sb[:, ec:ec + 1],
                            op0=OP.mult, op1=OP.add)
                        qts[q0] = qs
                    else:
                        nc.vector.tensor_copy(out=kt_t[:, q0:q0 + qw],
                                              in_=ps[:, :qw])
            return kt_t, qts

        def emit_v_chunk(w_sb, sc, s0, sw):
            for eh in range(D // EH):
                ps = gen_ps.tile([P, 512], F32, tag="mm", name="v_ps")
                for dc in range(ND):
                    nc.tensor.matmul(
                        ps[:sw, :EH],
                        xT_sb[:, dc, s0:s0 + sw],
                        w_sb[:, dc, eh * EH:(eh + 1) * EH],
                        start=(dc == 0), stop=(dc == ND - 1))
                vh = V[:sw, sc, :].rearrange("p (h w) -> p h w", w=DH + 1)
                nc.vector.tensor_copy(
                    out=vh[:, eh * (EH // DH):(eh + 1) * (EH // DH), 0:DH],
                    in_=ps[:sw, :EH].rearrange("p (h w) -> p h w", w=DH))

        # pr-major emission (emission order IS program order under Tile):
        # each head-pair's K/Q projection is followed by that pair's
        # attention over ALL q-blocks, so the 6 projection units spread
        # across 18 ACT-bound attention units and ScalarE stays the pacer.
        # The V pass interleaves chunk-by-chunk with the very first pair so
        # exp work starts within ~20us of kernel start. Each q-block's
        # output projection is emitted right after its last pair.
        cns = [ctxn_pool.tile([P, NE, 512], CDT, tag="cn", name=f"cn{_q}")
               for _q in range(len(QBS))]
        # Software pipeline: each unit's ctx phase (PE-cheap, DVE-chained)
        # is emitted AFTER the NEXT unit's scores, so the in-order PE
        # stream always hands ACT its 12 exps before grinding the
        # previous unit's ctx/evictions -- otherwise ACT starves ~5us at
        # every q-block boundary.
        pending = []

        def flush_pending():
            while pending:
                st_, cn_ = pending.pop(0)
                phase2_ctx(st_, cn_)

        for pr in range(NE):
            kt_t, qts = emit_kq(pr, pre=pre0 if pr == 0 else None)
            for qi, (q0, qw) in enumerate(QBS):
                if pr == 0 and qi <= 2:
                    # Spread the V projection (23us of PE) over the first
                    # TWO units' k-chunk interleave: packed into one unit
                    # it paces that unit's exps at ~2.3us instead of
                    # ~1.05us and ACT idles ~15us. The deferred-ctx
                    # pipeline flushes ctx(unit0) only after unit1's
                    # scores, so V still completes in time.
                    st = phase2_begin(q0, qw, 0, kt_t, qts)
                    if qi == 0:
                        w_sb = xw.tile([P, ND, D], CDT, tag="w",
                                       name="w_sb")
                        for dc in range(ND):
                            nc.gpsimd.dma_start(
                                out=w_sb[:, dc, :],
                                in_=wvT[dc * P:(dc + 1) * P, :])
                    third_v = (NSC + 2) // 3
                    for kc in range(NSC):
                        if kc % 3 == 0:
                            sc = qi * third_v + kc // 3
                            if sc < NSC and sc < (qi + 1) * third_v:
                                (s0, sw) = SC[sc]
                                emit_v_chunk(w_sb, sc, s0, sw)
                        phase2_kc(st, kc)
                else:
                    st = phase2_scores(q0, qw, pr, kt_t, qts)
                pending.append((st, cns[qi]))
                ql0, qlw = QBS[-1]
                ov = (QBS[-2][0] + QBS[-2][1]) - ql0 if len(QBS) > 1 else 0
                if pr == NE - 2 and qi == 0:
                    # pairs 0..NE-3 have finished the last q-block's ctx by
                    # now (the deferred pipeline is one unit behind): emit
                    # the partial output accumulations here so they fill
                    # PE gaps across the last TWO pairs' stretches.
                    flush_pending()
                    q2parts = phase3_partials(ql0, qlw, cns[-1], s_off=ov)
                if pr == NE - 1:
                    flush_pending()
                    # Interleave: finish the PREVIOUS q-block's second half
                    # here so its PE work covers this block's normalization
                    # chain; the current block keeps its first half only.
                    nsc_q = len(_chunks(qw, P))
                    half = nsc_q // 2
                    if qi > 0:
                        pq0, pqw = QBS[qi - 1]
                        pn = len(_chunks(pqw, P))
                        phase3(pq0, pqw, cns[qi - 1],
                               s_sel=set(range(pn // 2, pn)))
                    if qi < len(QBS) - 1:
                        phase3(q0, qw, cns[qi], s_sel=set(range(half)))
                    else:
                        phase3_final(q0, qw, cns[qi], q2parts, s_off=ov)
                elif len(pending) > 3:
                    phase2_ctx(*pending.pop(0))


def build_nc(S=S_FULL, reps=1, hw_loop=False):
    nc = bacc.Bacc("TRN2", target_bir_lowering=False, debug=False,
                   enable_asserts=False, num_devices=1)
    xT = nc.dram_tensor("xT", [D, S], CDT, kind="ExternalInput").ap()
    wqT = nc.dram_tensor("wqT", [NE, P, D], CDT, kind="ExternalInput").ap()
    wkT = nc.dram_tensor("wkT", [NE, P, D], CDT, kind="ExternalInput").ap()
    wvT = nc.dram_tensor("wvT", [D, D], CDT, kind="ExternalInput").ap()
    woT = nc.dram_tensor("woT", [D, D], CDT, kind="ExternalInput").ap()
    bqs = nc.dram_tensor("bqs", [P, NE], F32, kind="ExternalInput").ap()
    ident = nc.dram_tensor("ident", [P, P], CDT, kind="ExternalInput").ap()
    out = nc.dram_tensor("out", [S, D], CDT, kind="ExternalOutput").ap()
    with tile.TileContext(nc) as tc:
        with ExitStack() as ctx:
            build_attention(tc, ctx, xT, wqT, wkT, wvT, woT, bqs, ident,
                            out, S, reps, hw_loop)
    nc.compile()
    return nc


_NC_CACHE = {}


def _get_nc(S=S_FULL, reps=1, hw_loop=False):
    key = (S, reps, hw_loop)
    if key not in _NC_CACHE:
        _NC_CACHE[key] = build_nc(S, reps, hw_loop)
    return _NC_CACHE[key]


def prep_inputs(x, Wq, bq, Wk, Wv, bv, Wo, bo):
    x = np.asarray(x, dtype=np.float32)
    Wq = np.asarray(Wq, dtype=np.float32)
    Wk = np.asarray(Wk, dtype=np.float32)
    Wv = np.asarray(Wv, dtype=np.float32)
    Wo = np.asarray(Wo, dtype=np.float32)
    bq = np.asarray(bq, dtype=np.float32)
    bv = np.asarray(bv, dtype=np.float32)
    bo = np.asarray(bo, dtype=np.float32)
    xT = np.ascontiguousarray(x.transpose(0, 2, 1)).astype(NPCDT)
    def _sw(wt):
        # [D, D] -> [NE, P, ND*P]: block-column ec, gathered over dc rows
        return np.ascontiguousarray(
            wt.reshape(ND, P, NE, P).transpose(2, 1, 0, 3).reshape(NE, P, D))
    base = {
        "wqT": _sw(Wq.T).astype(NPCDT),
        "wkT": _sw(Wk.T).astype(NPCDT),
        "wvT": np.ascontiguousarray(Wv.T).astype(NPCDT),
        "woT": np.ascontiguousarray(Wo.T).astype(NPCDT),
        "bqs": np.ascontiguousarray((SCALE * bq).reshape(NE, P).T),
        "ident": np.eye(P, dtype=np.float32).astype(NPCDT),
    }
    const_row = (bv @ Wo.T + bo).astype(np.float32)
    in_maps = [dict(base, xT=np.ascontiguousarray(xT[b])) for b in range(x.shape[0])]
    return in_maps, const_row


def kernel(x, Wq, bq, Wk, Wv, bv, Wo, bo):
    in_maps, const_row = prep_inputs(x, Wq, bq, Wk, Wv, bv, Wo, bo)
    nc = _get_nc(x.shape[1])
    res = bass_utils.run_bass_kernel_spmd(
        nc, in_maps, core_ids=list(range(len(in_maps))))
    out = np.stack([np.asarray(r["out"], dtype=np.float32)
                    for r in res.results])
    return (out + const_row[None, None, :]).astype(np.float32)

